# revision 4
# baseline (speedup 1.0000x reference)
"""CSPN (convolutional spatial propagation) Trainium2 kernel, v2.

Full inputs:  guidance [8, 8, 512, 512] f32, x [8, 1, 512, 512] f32.
Sharding: data-parallel over batch -- core b gets batch element b.

v2 redesign vs the halo-DMA baseline:
  * No halo rows at all.  h is [128, 4, PW]: partition p holds image rows
    4p..4p+3 (slots 0-3).  The 6 tap-rows whose input row lives on a
    neighbouring partition (slot0 di=-1, slot3 di=+1) are multiplied ON the
    partition that owns the h row, against one-time partition-shifted copies
    of the weights (gtc).  The PE re-aligns those products to the output
    partition with shifted-identity stationary matrices (free in both the
    cost model and -- via dual weight buffers -- nearly free on HW).
  * Product rows are split DVE 29 / Pool 7 per iteration (the real
    TRN2 ISA allows only plain tensor_tensor on Pool -- the faster
    TensorScalarPtr family is DVE-only).  Setup scale-muls all run
    on DVE in 2x mode (rr stored f16); PE p-state is kept warm with
    dummy-matmul keepalive blocks.
  * DVE products are merged into 5 wide instructions per iteration
    ([2-3, 3, 514] APs with double-broadcast h), cutting per-instruction
    overhead; Pool does one [3, 3, 514] instruction (slot 2).
  * Steady state is PE-bound at 36 matmul-columns x 512 per iteration
    (~7.7 us); DVE ~7.5 us, Pool ~6.5 us, ACT ~2.5 us.

Numerics are identical to the baseline: fp16 weights/products, fp32 PSUM
accumulation, center gate gt4 = 1 - sum(fp16-rounded gt_k), h rescaled by
0.5/iter to stay in fp16 range, un-scaled by 2^24 in the final fp32 pass.
"""

import sys

sys.path.insert(0, "/opt/trn_rl_repo")

import numpy as np

import concourse.bass as bass
from concourse import mybir
from concourse.bass_utils import run_bass_kernel_spmd
from concourse.alu_op_type import AluOpType

F16 = mybir.dt.float16
F32 = mybir.dt.float32
U16 = mybir.dt.uint16
AF = mybir.ActivationFunctionType

N_CORES = 8
H, W = 512, 512
NS = 4            # row-slots per partition
PW = 520          # f16 elements per row slot (514 used + pad)
NITER = 24
RESCALE = 0.5
# tap k = (di+1)*3 + (dj+1); k=4 is the center gate
OFFS = [(k // 3 - 1, k % 3 - 1) for k in range(9)]
# guidance channel for tap k (center has none)
CH_FOR_K = [0, 1, 2, 3, None, 4, 5, 6, 7]
# setup scale-muls: all 8 on DVE (rr is f16 so they run in 2x mode);
# the real TRN2 ISA only allows plain tensor_tensor on Pool, whose 0.42
# efficiency makes it useless for the small setup muls
DVE_SETUP_KS = (0, 1, 2, 3, 5, 6, 7, 8)
# eviction order within a cycle (follows the PE stop-group order) and the
# 1-based position of each slot's eviction in that order
EVORD = [1, 0, 2, 3]
EVPOS = {1: 1, 0: 2, 2: 3, 3: 4}
# DVE product-instruction count at t=0 (9: split center groups)
N_T0_MUL = 10
N_KEEPALIVE = 60


def build_program(niter=NITER):
    nc = bass.Bass("TRN2", target_bir_lowering=False, debug=False)

    g_dram = nc.dram_tensor("guidance", [8, H, W], F32, kind="ExternalInput")
    x_dram = nc.dram_tensor("x", [1, H, W], F32, kind="ExternalInput")
    o_dram = nc.dram_tensor("out", [1, H, W], F32, kind="ExternalOutput")

    h0 = nc.alloc_sbuf_tensor("h0", [128, NS, PW], F16)
    h1 = nc.alloc_sbuf_tensor("h1", [128, NS, PW], F16)
    # weights / products indexed [kg, kj, slot, col] with tap k = 3*kg + kj
    gt = nc.alloc_sbuf_tensor("gt", [128, 3, 3, NS, PW], F16)
    pr = nc.alloc_sbuf_tensor("pr", [128, 3, 3, NS, PW], F16)
    # slot-2 products, double-buffered so Pool(t+1) never waits on PE2(t)
    pr2 = nc.alloc_sbuf_tensor("pr2", [128, 2, 3, 3, PW], F16)
    # cross rows: 0-2 = slot0 taps k=0,1,2 shifted up (gtc[p] = gt[p+1]);
    #             3-5 = slot3 taps k=6,7,8 shifted down (gtc[p] = gt[p-1])
    gtc = nc.alloc_sbuf_tensor("gtc", [128, 6, PW], F16)
    prc = nc.alloc_sbuf_tensor("prc", [128, 6, PW], F16)
    graw = nc.alloc_sbuf_tensor("graw", [128, 8, NS, PW], F16)
    gabs = nc.alloc_sbuf_tensor("gabs", [128, 2, 8, PW], F16)
    asb = nc.alloc_sbuf_tensor("asb", [128, NS, PW], F32)   # ln(A); out stage
    rr = nc.alloc_sbuf_tensor("rr", [128, NS, PW], F16)     # r = 1/A
    ident = nc.alloc_sbuf_tensor("ident", [128, 128], F16)
    wup = nc.alloc_sbuf_tensor("wup", [128, 128], F16)      # psum[p] += x[p-1]
    wdn = nc.alloc_sbuf_tensor("wdn", [128, 128], F16)      # psum[p] += x[p+1]
    c_eps = nc.alloc_sbuf_tensor("c_eps", [128, 1], F32)

    psum = [nc.alloc_psum_tensor(f"pg{g}s{s}", [128, W], F32)
            for g in range(2) for s in range(NS)]

    def pg(g, s):
        return psum[g * NS + s].ap()

    hb = [h0, h1]

    s_hz = nc.alloc_semaphore("s_hz")      # DVE memsets done
    s_id = nc.alloc_semaphore("s_id")      # stationaries built (3)
    s_x = nc.alloc_semaphore("s_x")        # x DMA (+16)
    s_gs = [nc.alloc_semaphore(f"s_g{i}") for i in range(NS)]
    s_abs = nc.alloc_semaphore("s_abs")    # DVE abs per slot
    s_apex = nc.alloc_semaphore("s_apex")  # PE A-sum per slot
    s_ln = nc.alloc_semaphore("s_ln")      # ACT ln per slot
    s_rexp = nc.alloc_semaphore("s_rexp")  # ACT exp per slot
    s_gtd = nc.alloc_semaphore("s_gtd")    # DVE setup muls (5/slot)
    s_gtp = nc.alloc_semaphore("s_gtp")    # Pool setup muls (3/slot)
    s_cpe = nc.alloc_semaphore("s_cpe")    # PE center sum per slot
    s_cev = nc.alloc_semaphore("s_cev")    # ACT center evict per slot
    s_gtcu = nc.alloc_semaphore("s_gtcu")  # gtc up-shift DMA (+16)
    s_gtcd = nc.alloc_semaphore("s_gtcd")  # gtc down-shift DMA (+16)
    s_mul = nc.alloc_semaphore("s_mul")    # DVE iter muls (+5/iter)
    s_mulp = nc.alloc_semaphore("s_mulp")  # Pool iter muls (+1/iter)
    s_pe = nc.alloc_semaphore("s_pe")      # PE tap-sum per slot (+4/iter)
    s_ev = nc.alloc_semaphore("s_ev")      # ACT evict per slot (+4/iter)
    s_fin = nc.alloc_semaphore("s_fin")    # final ACT rescale (4)
    s_out = nc.alloc_semaphore("s_out")    # output DMA

    N_MEMSET = 6

    def gk(k):
        return k // 3, k % 3

    def emit_dve_gtmuls(v, s):
        v.wait_ge(s_hz, 14)   # gt pad memsets retired (same-engine WAW)
        v.wait_ge(s_rexp, s + 1)
        for k in DVE_SETUP_KS:
            kg_, kj_ = gk(k)
            dj = OFFS[k][1]
            v.tensor_tensor(
                out=gt.ap()[:, kg_, kj_, s, 1 + dj:513 + dj],
                in0=graw.ap()[:, CH_FOR_K[k], s, 1:513],
                in1=rr.ap()[:, s, 1:513],
                op=AluOpType.mult,
            ).then_inc(s_gtd, 1)

    def h_bcast(h, lo, hi, width):
        """h slots [lo:hi] broadcast over the 3-tap (kj) dim."""
        n = hi - lo
        return (h.ap()[:, lo:hi, 0:514]
                .unsqueeze(2).broadcast_to([width, n, 3, 514]))

    with nc.Block() as block:

        # ---------------- GPSIMD/Pool: DMAs, stationaries, products ----------
        @block.gpsimd
        def _(gp):
            # guidance slot 0 first -- needs no memsets, so it starts at t=0
            g_in0 = bass.AP(g_dram, 0, [[4 * W, 128], [H * W, 8], [1, W]])
            gp.dma_start(graw.ap()[:, :, 0, 1:513], g_in0).then_inc(s_gs[0], 16)
            gp.wait_ge(s_hz, 3)
            for w_t, base in ((ident, 0), (wup, 1), (wdn, -1)):
                gp.affine_select(
                    out=w_t.ap(), in_=w_t.ap(),
                    compare_op=AluOpType.not_equal, fill=1.0, base=base,
                    pattern=[[-1, 128]], channel_multiplier=1,
                ).then_inc(s_id, 1)
            for s in range(1, NS):
                g_in = bass.AP(g_dram, s * W,
                               [[4 * W, 128], [H * W, 8], [1, W]])
                gp.dma_start(graw.ap()[:, :, s, 1:513], g_in).then_inc(s_gs[s], 16)
            # x last: it is not needed until the first products (~cev0 time)
            gp.wait_ge(s_hz, 5)
            x_in = bass.AP(x_dram, 0, [[4 * W, 128], [W, NS], [1, W]])
            gp.dma_start(h0.ap()[:, :, 1:513], x_in).then_inc(s_x, 16)
            # iteration products: slot 2, one stt per tap-triple (kg) so each
            # starts as soon as its h slot (1+kg) is evicted
            def pool_kg(t, hcur, kg_):
                gp.tensor_tensor(
                    out=pr2.ap()[:, t % 2, kg_, :, 0:514],
                    in0=gt.ap()[:, kg_, :, 2, 0:514],
                    in1=(hcur.ap()[:, 1 + kg_:2 + kg_, 0:514]
                         .broadcast_to([128, 3, 514])),
                    op=AluOpType.mult,
                ).then_inc(s_mulp, 1)

            def pool_k6(t, hcur):
                gp.tensor_tensor(
                    out=pr2.ap()[:, t % 2, 2, 0, 0:514],
                    in0=gt.ap()[:, 2, 0, 2, 0:514],
                    in1=hcur.ap()[:, 3, 0:514],
                    op=AluOpType.mult,
                ).then_inc(s_mulp, 1)

            # t=0: kg1 holds the center tap (needs cev2); run it last
            gp.wait_ge(s_gtd, 24)
            gp.wait_ge(s_x, 16)
            pool_kg(0, h0, 0)
            pool_k6(0, h0)
            gp.wait_ge(s_cev, 3)
            pool_kg(0, h0, 1)
            for t in range(1, niter):
                hcur = hb[t % 2]
                gp.wait_ge(s_ev, 4 * (t - 1) + EVPOS[1])
                if t >= 2:
                    gp.wait_ge(s_pe, 6 * (t - 2) + 5)
                pool_kg(t, hcur, 0)
                gp.wait_ge(s_ev, 4 * (t - 1) + EVPOS[2])
                pool_kg(t, hcur, 1)
                gp.wait_ge(s_ev, 4 * (t - 1) + EVPOS[3])
                pool_k6(t, hcur)
            gp.wait_ge(s_out, 64)

        # ---------------- DVE: memsets, abs, setup muls, products ------------
        @block.vector
        def _(v):
            v.memset(ident.ap(), 0.0).then_inc(s_hz, 1)
            v.memset(wup.ap(), 0.0).then_inc(s_hz, 1)
            v.memset(wdn.ap(), 0.0).then_inc(s_hz, 1)
            # h pad columns only; bodies are written by the x DMA (h0) and
            # by evictions (h1) before any read
            v.memset(h0.ap()[:, :, 0:1], 0.0).then_inc(s_hz, 1)
            v.memset(h0.ap()[:, :, 513:520], 0.0).then_inc(s_hz, 1)
            v.memset(c_eps.ap(), 1e-8).then_inc(s_hz, 1)
            v.memset(h1.ap()[:, :, 0:1], 0.0).then_inc(s_hz, 1)
            v.memset(h1.ap()[:, :, 513:520], 0.0).then_inc(s_hz, 1)
            # abs per slot as guidance DMAs land; pad memsets fill the gaps
            # (their consumers are transitively gated through s_gtd/s_mul)
            for s in range(NS):
                v.wait_ge(s_gs[s], 16)
                if s >= 2:
                    v.wait_ge(s_apex, s - 1)  # gabs[s%2] free again
                v.tensor_scalar(
                    out=gabs.ap()[:, s % 2, :, 1:513].bitcast(U16),
                    in0=graw.ap()[:, :, s, 1:513].bitcast(U16),
                    scalar1=0x7FFF, scalar2=None, op0=AluOpType.bitwise_and,
                ).then_inc(s_abs, 1)
                if s == 0:
                    for kg_ in range(3):
                        v.memset(gt.ap()[:, kg_, :, :, 0:2], 0.0).then_inc(s_hz, 1)
                        v.memset(gt.ap()[:, kg_, :, :, 512:520], 0.0).then_inc(s_hz, 1)
                else:
                    # slot s-1 scale-muls, interleaved so they start as soon
                    # as exp(s-1) lands instead of after all four abs passes
                    emit_dve_gtmuls(v, s - 1)
            emit_dve_gtmuls(v, 3)
            # iteration products: 7 instructions per iteration, ordered by
            # gate availability then consumer deadline.  PE window order:
            # 0a'(k678), slot1, 0a''(k345), 0b(cross), slot2, slot3.
            # t=0 uses a different order and precise setup gates (only the
            # tap-groups containing the center k=4 need the center-evict).
            def p_s0kg2(t, hcur):
                v.tensor_tensor(
                    out=pr.ap()[:, 2, :, 0, 0:514],
                    in0=gt.ap()[:, 2, :, 0, 0:514],
                    in1=(hcur.ap()[:, 1:2, 0:514]
                         .broadcast_to([128, 3, 514])),
                    op=AluOpType.mult,
                ).then_inc(s_mul, 1)

            def p_s1kg01(t, hcur):
                v.tensor_tensor(
                    out=pr.ap()[:, 0:2, :, 1, 0:514],
                    in0=gt.ap()[:, 0:2, :, 1, 0:514],
                    in1=h_bcast(hcur, 0, 2, 128), op=AluOpType.mult,
                ).then_inc(s_mul, 1)

            def p_s1kg2(t, hcur):
                v.tensor_tensor(
                    out=pr.ap()[:, 2, :, 1, 0:514],
                    in0=gt.ap()[:, 2, :, 1, 0:514],
                    in1=(hcur.ap()[:, 2:3, 0:514]
                         .broadcast_to([128, 3, 514])),
                    op=AluOpType.mult,
                ).then_inc(s_mul, 1)

            def p_s0kg1(t, hcur):
                v.tensor_tensor(
                    out=pr.ap()[:, 1, :, 0, 0:514],
                    in0=gt.ap()[:, 1, :, 0, 0:514],
                    in1=(hcur.ap()[:, 0:1, 0:514]
                         .broadcast_to([128, 3, 514])),
                    op=AluOpType.mult,
                ).then_inc(s_mul, 1)

            def p_up3(t, hcur):
                v.tensor_tensor(
                    out=prc.ap()[:, 0:3, 0:514],
                    in0=gtc.ap()[:, 0:3, 0:514],
                    in1=(hcur.ap()[:, 3:4, 0:514]
                         .broadcast_to([128, 3, 514])),
                    op=AluOpType.mult,
                ).then_inc(s_mul, 1)

            def p_s3kg01(t, hcur):
                v.tensor_tensor(
                    out=pr.ap()[:, 0:2, :, 3, 0:514],
                    in0=gt.ap()[:, 0:2, :, 3, 0:514],
                    in1=h_bcast(hcur, 2, 4, 128), op=AluOpType.mult,
                ).then_inc(s_mul, 1)

            def p_s2k78(t, hcur):
                v.tensor_tensor(
                    out=pr2.ap()[:, t % 2, 2, 1:3, 0:514],
                    in0=gt.ap()[:, 2, 1:3, 2, 0:514],
                    in1=(hcur.ap()[:, 3:4, 0:514]
                         .broadcast_to([128, 2, 514])),
                    op=AluOpType.mult,
                ).then_inc(s_mul, 1)

            def p_dn3(t, hcur):
                v.tensor_tensor(
                    out=prc.ap()[:, 3:6, 0:514],
                    in0=gtc.ap()[:, 3:6, 0:514],
                    in1=(hcur.ap()[:, 0:1, 0:514]
                         .broadcast_to([128, 3, 514])),
                    op=AluOpType.mult,
                ).then_inc(s_mul, 1)

            # t=0: split the center-bearing tap-groups so only three tail
            # instructions wait on center-evicts; everything else is gated by
            # the scale-muls / gtc shifts alone
            def p_kg(sg, kg_, hslot):
                v.tensor_tensor(
                    out=pr.ap()[:, kg_, :, sg, 0:514],
                    in0=gt.ap()[:, kg_, :, sg, 0:514],
                    in1=(h0.ap()[:, hslot:hslot + 1, 0:514]
                         .broadcast_to([128, 3, 514])),
                    op=AluOpType.mult,
                ).then_inc(s_mul, 1)

            v.wait_ge(s_gtd, 8)
            v.wait_ge(s_x, 16)
            p_s0kg2(0, h0)            # 1
            v.wait_ge(s_gtd, 16)
            p_kg(1, 0, 0)             # 2: slot1 k0-2
            p_s1kg2(0, h0)            # 3
            v.wait_ge(s_gtcu, 32)
            p_up3(0, h0)              # 4
            v.wait_ge(s_gtcd, 32)
            p_dn3(0, h0)              # 5
            v.wait_ge(s_gtd, 32)
            p_kg(3, 0, 2)             # 6: slot3 k0-2
            p_s2k78(0, h0)            # 7: slot2 k7,k8
            v.wait_ge(s_cev, 1)
            p_kg(0, 1, 0)             # 8: slot0 k3-5 (center)
            v.wait_ge(s_cev, 2)
            p_kg(1, 1, 1)             # 9: slot1 k3-5 (center)
            v.wait_ge(s_cev, 4)
            p_kg(3, 1, 3)             # 10: slot3 k3-5 (center)
            # t>=1: steady-state order and gates
            for t in range(1, niter):
                hcur = hb[t % 2]
                v.wait_ge(s_ev, 4 * (t - 1) + 1)
                v.wait_ge(s_pe, 6 * (t - 1) + 1)
                p_s0kg2(t, hcur)
                v.wait_ge(s_ev, 4 * (t - 1) + 2)
                v.wait_ge(s_pe, 6 * (t - 1) + 2)
                p_s1kg01(t, hcur)
                v.wait_ge(s_ev, 4 * (t - 1) + 3)
                p_s1kg2(t, hcur)
                v.wait_ge(s_pe, 6 * (t - 1) + 3)
                p_s0kg1(t, hcur)
                v.wait_ge(s_ev, 4 * (t - 1) + 4)
                v.wait_ge(s_pe, 6 * (t - 1) + 4)
                p_up3(t, hcur)
                if t >= 2:
                    v.wait_ge(s_pe, 6 * (t - 2) + 5)   # pr2 parity WAR
                p_s2k78(t, hcur)
                v.wait_ge(s_pe, 6 * (t - 1) + 6)
                p_s3kg01(t, hcur)
                p_dn3(t, hcur)

        # ---------------- PE: setup sums + iteration tap-sums ----------------
        @block.tensor
        def _(pe):
            pe.wait_ge(s_id, 3)
            # warm-up: ramp the PE p-state past the 3us threshold on dummy
            # 128-col matmuls so the first A-sum runs at full clock
            for _ in range(30):
                pe.matmul(pg(1, 0)[:, 0:128], ident.ap(), ident.ap(),
                          start=True, stop=True, skip_group_check=True)
            # A = sum_ch |g_ch| into psum group 0
            for s in range(NS):
                pe.wait_ge(s_abs, s + 1)
                for ch in range(8):
                    inst = pe.matmul(
                        pg(0, s)[:, 0:512], ident.ap(),
                        gabs.ap()[:, s % 2, ch, 1:513],
                        start=(ch == 0), stop=(ch == 7),
                    )
                    if ch == 7:
                        inst.then_inc(s_apex, 1)
            # center gate: sum of fp16-rounded weights into psum group 1
            for s in range(NS):
                pe.wait_ge(s_gtd, len(DVE_SETUP_KS) * (s + 1))
                done = 0
                for k in range(9):
                    if k == 4:
                        continue
                    kg_, kj_ = gk(k)
                    dj = OFFS[k][1]
                    inst = pe.matmul(
                        pg(1, s)[:, 0:512], ident.ap(),
                        gt.ap()[:, kg_, kj_, s, 1 + dj:513 + dj],
                        start=(done == 0), stop=(done == 7),
                    )
                    done += 1
                    if done == 8:
                        inst.then_inc(s_cpe, 1)
            # keepalive: bridge the idle gap between setup and the first
            # window so the p-state ramp survives into T0's matmuls.
            # pg(0,0)'s only setup reader (ln0) is gated done via s_ln.
            pe.wait_ge(s_ln, 1)
            for _ in range(N_KEEPALIVE):
                pe.matmul(pg(0, 0)[:, 0:128], ident.ap(), ident.ap(),
                          start=True, stop=True, skip_group_check=True)
            # iterations: window order 0a'(k678, start psum0), slot1,
            # 0a''(k345), 0b(cross, stop psum0), slot2, slot3;
            # s_pe +1 after each of the 6 groups
            for t in range(niter):
                # t=0 DVE emission order differs; map each group to the
                # s_mul count of its last-needed product
                mw = ([1, 9, 8, 4, 7, 10] if t == 0
                      else [N_T0_MUL + 8 * (t - 1) + 1, N_T0_MUL + 8 * (t - 1) + 3,
                            N_T0_MUL + 8 * (t - 1) + 4, N_T0_MUL + 8 * (t - 1) + 5,
                            N_T0_MUL + 8 * (t - 1) + 6, N_T0_MUL + 8 * (t - 1) + 8])
                groups = [
                    # (psum slot, taps, s_mul wait, s_mulp wait, start, stop)
                    (0, [(k, ident, None) for k in range(6, 9)],
                     mw[0], None, True, False),
                    (1, [(k, ident, None) for k in range(9)],
                     mw[1], None, True, True),
                    (0, [(k, ident, None) for k in range(3, 6)],
                     mw[2], None, False, False),
                    (0, [(k, wup, k) for k in range(3)],
                     mw[3], None, False, True),
                    (2, [(k, ident, None) for k in range(9)],
                     mw[4], 3 * t + 3, True, True),
                    (3, [(k, ident, None) for k in range(6)]
                        + [(k, wdn, 3 + k - 6) for k in range(6, 9)],
                     mw[5], None, True, True),
                ]
                for sg, taps, mw, mpw, st, sp in groups:
                    if mw is not None:
                        pe.wait_ge(s_mul, mw)
                    if mpw is not None:
                        pe.wait_ge(s_mulp, mpw)
                    if st:
                        # psum(sg, t%2) must be drained by its previous reader
                        if t == 0:
                            pe.wait_ge(s_ln, sg + 1)
                        elif t == 1:
                            pe.wait_ge(s_cev, sg + 1)
                        else:
                            pe.wait_ge(s_ev, 4 * (t - 2) + EVPOS[sg])
                    n = len(taps)
                    for j, (k, w_t, crow) in enumerate(taps):
                        kg_, kj_ = gk(k)
                        dj = OFFS[k][1]
                        if sg == 2:
                            mv = pr2.ap()[:, t % 2, kg_, kj_, 1 + dj:513 + dj]
                        elif crow is None:
                            mv = pr.ap()[:, kg_, kj_, sg, 1 + dj:513 + dj]
                        else:
                            mv = prc.ap()[:, crow, 1 + dj:513 + dj]
                        inst = pe.matmul(
                            pg(t % 2, sg)[:, 0:512], w_t.ap(), mv,
                            start=(st and j == 0), stop=(sp and j == n - 1),
                            skip_group_check=True,
                        )
                        if j == n - 1:
                            inst.then_inc(s_pe, 1)

        # ---------------- ACT: ln/exp normalization, evictions ---------------
        @block.scalar
        def _(sc):
            sc.wait_ge(s_hz, 6)   # c_eps ready
            for s in range(NS):
                sc.wait_ge(s_apex, s + 1)
                sc.activation(
                    asb.ap()[:, s, 1:513], pg(0, s)[:, 0:512], AF.Ln,
                    bias=c_eps.ap(),
                ).then_inc(s_ln, 1)
                sc.wait_ge(s_ln, s + 1)
                sc.activation(
                    rr.ap()[:, s, 1:513], asb.ap()[:, s, 1:513], AF.Exp,
                    scale=-1.0,
                ).then_inc(s_rexp, 1)
            for s in range(NS):
                sc.wait_ge(s_cpe, s + 1)
                sc.activation(
                    gt.ap()[:, 1, 1, s, 1:513], pg(1, s)[:, 0:512], AF.Identity,
                    bias=1.0, scale=-1.0,
                ).then_inc(s_cev, 1)
            # eviction order [1, 0, 2, 3] matches PE stop-group order
            pe_stop = {1: 2, 0: 4, 2: 5, 3: 6}
            # DVE cycle-(t-1) readers of h slot sg (last reading instr index)
            war_mul = {0: 8, 1: 2, 2: 7, 3: 7}
            war_mulp = {1: 1, 2: 2, 3: 3}
            for t in range(niter):
                hnext = hb[(t + 1) % 2]
                last = (t == niter - 1)
                for sg in EVORD:
                    sc.wait_ge(s_pe, 6 * t + pe_stop[sg])
                    if t == 0 and sg == EVORD[0]:
                        sc.wait_ge(s_hz, 8)   # h1 pad memsets done
                    if t > 0:
                        # h_next WAR: cycle t-1 readers of this buffer+slot
                        sc.wait_ge(s_mul, N_T0_MUL if t == 1
                                   else N_T0_MUL + 8 * (t - 2) + war_mul[sg])
                        if sg in war_mulp:
                            sc.wait_ge(s_mulp, 3 * (t - 1) + war_mulp[sg])
                    if last:
                        # final eviction straight to fp32 at true magnitude
                        # (undoes the 23 earlier rescales plus this one)
                        sc.activation(
                            asb.ap()[:, sg, 1:513],
                            pg(t % 2, sg)[:, 0:512], AF.Copy,
                            scale=RESCALE * float(1.0 / RESCALE) ** niter,
                        ).then_inc(s_fin, 1)
                    else:
                        sc.activation(
                            hnext.ap()[:, sg, 1:513],
                            pg(t % 2, sg)[:, 0:512], AF.Copy,
                            scale=RESCALE,
                        ).then_inc(s_ev, 1)

        # ---------------- SYNC: gtc shift DMAs + output ----------------------
        @block.sync
        def _(sy):
            # cross weights: one-time partition shifts of the normalized gt
            sy.wait_ge(s_gtd, len(DVE_SETUP_KS))
            sy.dma_start(gtc.ap()[0:127, 0:3, 0:514],
                         gt.ap()[1:128, 0, :, 0, 0:514]).then_inc(s_gtcu, 16)
            sy.dma_start(gtc.ap()[127:128, 0:3, 0:514],
                         gt.ap()[0:1, 0, :, 0, 0:514]).then_inc(s_gtcu, 16)
            sy.wait_ge(s_gtd, len(DVE_SETUP_KS) * NS)
            sy.dma_start(gtc.ap()[1:128, 3:6, 0:514],
                         gt.ap()[0:127, 2, :, 3, 0:514]).then_inc(s_gtcd, 16)
            sy.dma_start(gtc.ap()[0:1, 3:6, 0:514],
                         gt.ap()[127:128, 2, :, 3, 0:514]).then_inc(s_gtcd, 16)
            # per-slot output DMAs, each as soon as its final eviction lands
            for i, sg in enumerate(EVORD):
                sy.wait_ge(s_fin, i + 1)
                o_out = bass.AP(o_dram, sg * W, [[4 * W, 128], [1, W]])
                sy.dma_start(o_out, asb.ap()[:, sg, 1:513]).then_inc(s_out, 16)

    return nc


_NC_CACHE = {}


def kernel(guidance: np.ndarray, x: np.ndarray) -> np.ndarray:
    """guidance [8,8,512,512] f32, x [8,1,512,512] f32 -> [8,1,512,512] f32."""
    guidance = np.ascontiguousarray(np.asarray(guidance, dtype=np.float32))
    x = np.ascontiguousarray(np.asarray(x, dtype=np.float32))
    if "nc" not in _NC_CACHE:
        _NC_CACHE["nc"] = build_program()
    nc = _NC_CACHE["nc"]
    in_maps = [
        {"guidance": guidance[b], "x": x[b].reshape(1, H, W)}
        for b in range(N_CORES)
    ]
    res = run_bass_kernel_spmd(nc, in_maps, core_ids=list(range(N_CORES)))
    out = np.stack([res.results[b]["out"] for b in range(N_CORES)], axis=0)
    return out.astype(np.float32)


# revision 5
# speedup vs baseline: 1.0076x; 1.0076x over previous
"""CSPN (convolutional spatial propagation) Trainium2 kernel, v2.

Full inputs:  guidance [8, 8, 512, 512] f32, x [8, 1, 512, 512] f32.
Sharding: data-parallel over batch -- core b gets batch element b.

v2 redesign vs the halo-DMA baseline:
  * No halo rows at all.  h is [128, 4, PW]: partition p holds image rows
    4p..4p+3 (slots 0-3).  The 6 tap-rows whose input row lives on a
    neighbouring partition (slot0 di=-1, slot3 di=+1) are multiplied ON the
    partition that owns the h row, against one-time partition-shifted copies
    of the weights (gtc).  The PE re-aligns those products to the output
    partition with shifted-identity stationary matrices (free in both the
    cost model and -- via dual weight buffers -- nearly free on HW).
  * Product rows are split DVE 29 / Pool 7 per iteration (the real
    TRN2 ISA allows only plain tensor_tensor on Pool -- the faster
    TensorScalarPtr family is DVE-only).  Setup scale-muls all run
    on DVE in 2x mode (rr stored f16); PE p-state is kept warm with
    dummy-matmul keepalive blocks.
  * DVE products are merged into 5 wide instructions per iteration
    ([2-3, 3, 514] APs with double-broadcast h), cutting per-instruction
    overhead; Pool does one [3, 3, 514] instruction (slot 2).
  * Steady state is PE-bound at 36 matmul-columns x 512 per iteration
    (~7.7 us); DVE ~7.5 us, Pool ~6.5 us, ACT ~2.5 us.

Numerics are identical to the baseline: fp16 weights/products, fp32 PSUM
accumulation, center gate gt4 = 1 - sum(fp16-rounded gt_k), h rescaled by
0.5/iter to stay in fp16 range, un-scaled by 2^24 in the final fp32 pass.
"""

import sys

sys.path.insert(0, "/opt/trn_rl_repo")

import numpy as np

import concourse.bass as bass
from concourse import mybir
from concourse.bass_utils import run_bass_kernel_spmd
from concourse.alu_op_type import AluOpType

F16 = mybir.dt.float16
F32 = mybir.dt.float32
U16 = mybir.dt.uint16
AF = mybir.ActivationFunctionType

N_CORES = 8
H, W = 512, 512
NS = 4            # row-slots per partition
PW = 520          # f16 elements per row slot (514 used + pad)
NITER = 24
RESCALE = 0.5
# tap k = (di+1)*3 + (dj+1); k=4 is the center gate
OFFS = [(k // 3 - 1, k % 3 - 1) for k in range(9)]
# guidance channel for tap k (center has none)
CH_FOR_K = [0, 1, 2, 3, None, 4, 5, 6, 7]
# setup scale-muls: 6 on DVE (rr f16 -> 2x mode), 2 on the otherwise-idle
# Pool via plain tensor_tensor (the only mult op the real ISA allows there)
DVE_SETUP_KS = (0, 1, 2, 3, 5, 8)
POOL_SETUP_KS = (6, 7)
# eviction order within a cycle (follows the PE stop-group order) and the
# 1-based position of each slot's eviction in that order
EVORD = [1, 0, 2, 3]
EVPOS = {1: 1, 0: 2, 2: 3, 3: 4}
# DVE product-instruction count at t=0 (9: split center groups)
N_T0_MUL = 10
N_KEEPALIVE = 60


def build_program(niter=NITER):
    nc = bass.Bass("TRN2", target_bir_lowering=False, debug=False)

    g_dram = nc.dram_tensor("guidance", [8, H, W], F32, kind="ExternalInput")
    x_dram = nc.dram_tensor("x", [1, H, W], F32, kind="ExternalInput")
    o_dram = nc.dram_tensor("out", [1, H, W], F32, kind="ExternalOutput")

    h0 = nc.alloc_sbuf_tensor("h0", [128, NS, PW], F16)
    h1 = nc.alloc_sbuf_tensor("h1", [128, NS, PW], F16)
    # weights / products indexed [kg, kj, slot, col] with tap k = 3*kg + kj
    gt = nc.alloc_sbuf_tensor("gt", [128, 3, 3, NS, PW], F16)
    pr = nc.alloc_sbuf_tensor("pr", [128, 3, 3, NS, PW], F16)
    # slot-2 products, double-buffered so Pool(t+1) never waits on PE2(t)
    pr2 = nc.alloc_sbuf_tensor("pr2", [128, 2, 3, 3, PW], F16)
    # cross rows: 0-2 = slot0 taps k=0,1,2 shifted up (gtc[p] = gt[p+1]);
    #             3-5 = slot3 taps k=6,7,8 shifted down (gtc[p] = gt[p-1])
    gtc = nc.alloc_sbuf_tensor("gtc", [128, 6, PW], F16)
    prc = nc.alloc_sbuf_tensor("prc", [128, 6, PW], F16)
    graw = nc.alloc_sbuf_tensor("graw", [128, 8, NS, PW], F16)
    gabs = nc.alloc_sbuf_tensor("gabs", [128, 2, 8, PW], F16)
    asb = nc.alloc_sbuf_tensor("asb", [128, NS, PW], F32)   # ln(A); out stage
    rr = nc.alloc_sbuf_tensor("rr", [128, NS, PW], F16)     # r = 1/A
    ident = nc.alloc_sbuf_tensor("ident", [128, 128], F16)
    wup = nc.alloc_sbuf_tensor("wup", [128, 128], F16)      # psum[p] += x[p-1]
    wdn = nc.alloc_sbuf_tensor("wdn", [128, 128], F16)      # psum[p] += x[p+1]
    c_eps = nc.alloc_sbuf_tensor("c_eps", [128, 1], F32)

    psum = [nc.alloc_psum_tensor(f"pg{g}s{s}", [128, W], F32)
            for g in range(2) for s in range(NS)]

    def pg(g, s):
        return psum[g * NS + s].ap()

    hb = [h0, h1]

    s_hz = nc.alloc_semaphore("s_hz")      # DVE memsets done
    s_id = nc.alloc_semaphore("s_id")      # stationaries built (3)
    s_x = nc.alloc_semaphore("s_x")        # x DMA (+16)
    s_gs = [nc.alloc_semaphore(f"s_g{i}") for i in range(NS)]
    s_abs = nc.alloc_semaphore("s_abs")    # DVE abs per slot
    s_apex = nc.alloc_semaphore("s_apex")  # PE A-sum per slot
    s_ln = nc.alloc_semaphore("s_ln")      # ACT ln per slot
    s_rexp = nc.alloc_semaphore("s_rexp")  # ACT exp per slot
    s_gtd = nc.alloc_semaphore("s_gtd")    # DVE setup muls (5/slot)
    s_gtp = nc.alloc_semaphore("s_gtp")    # Pool setup muls (3/slot)
    s_cpe = nc.alloc_semaphore("s_cpe")    # PE center sum per slot
    s_cev = nc.alloc_semaphore("s_cev")    # ACT center evict per slot
    s_gtcu = nc.alloc_semaphore("s_gtcu")  # gtc up-shift DMA (+16)
    s_gtcd = nc.alloc_semaphore("s_gtcd")  # gtc down-shift DMA (+16)
    s_mul = nc.alloc_semaphore("s_mul")    # DVE iter muls (+5/iter)
    s_mulp = nc.alloc_semaphore("s_mulp")  # Pool iter muls (+1/iter)
    s_pe = nc.alloc_semaphore("s_pe")      # PE tap-sum per slot (+4/iter)
    s_ev = nc.alloc_semaphore("s_ev")      # ACT evict per slot (+4/iter)
    s_fin = nc.alloc_semaphore("s_fin")    # final ACT rescale (4)
    s_out = nc.alloc_semaphore("s_out")    # output DMA

    N_MEMSET = 6

    def gk(k):
        return k // 3, k % 3

    def emit_dve_gtmuls(v, s):
        v.wait_ge(s_hz, 14)   # gt pad memsets retired (same-engine WAW)
        v.wait_ge(s_rexp, s + 1)
        for k in DVE_SETUP_KS:
            kg_, kj_ = gk(k)
            dj = OFFS[k][1]
            v.tensor_tensor(
                out=gt.ap()[:, kg_, kj_, s, 1 + dj:513 + dj],
                in0=graw.ap()[:, CH_FOR_K[k], s, 1:513],
                in1=rr.ap()[:, s, 1:513],
                op=AluOpType.mult,
            ).then_inc(s_gtd, 1)

    def h_bcast(h, lo, hi, width):
        """h slots [lo:hi] broadcast over the 3-tap (kj) dim."""
        n = hi - lo
        return (h.ap()[:, lo:hi, 0:514]
                .unsqueeze(2).broadcast_to([width, n, 3, 514]))

    with nc.Block() as block:

        # ---------------- GPSIMD/Pool: DMAs, stationaries, products ----------
        @block.gpsimd
        def _(gp):
            # guidance slot 0 first -- needs no memsets, so it starts at t=0
            g_in0 = bass.AP(g_dram, 0, [[4 * W, 128], [H * W, 8], [1, W]])
            gp.dma_start(graw.ap()[:, :, 0, 1:513], g_in0).then_inc(s_gs[0], 16)
            gp.wait_ge(s_hz, 3)
            for w_t, base in ((ident, 0), (wup, 1), (wdn, -1)):
                gp.affine_select(
                    out=w_t.ap(), in_=w_t.ap(),
                    compare_op=AluOpType.not_equal, fill=1.0, base=base,
                    pattern=[[-1, 128]], channel_multiplier=1,
                ).then_inc(s_id, 1)
            for s in range(1, NS):
                g_in = bass.AP(g_dram, s * W,
                               [[4 * W, 128], [H * W, 8], [1, W]])
                gp.dma_start(graw.ap()[:, :, s, 1:513], g_in).then_inc(s_gs[s], 16)
            # x last: it is not needed until the first products (~cev0 time)
            gp.wait_ge(s_hz, 5)
            x_in = bass.AP(x_dram, 0, [[4 * W, 128], [W, NS], [1, W]])
            gp.dma_start(h0.ap()[:, :, 1:513], x_in).then_inc(s_x, 16)
            # iteration products: slot 2, one stt per tap-triple (kg) so each
            # starts as soon as its h slot (1+kg) is evicted
            def pool_kg(t, hcur, kg_):
                gp.tensor_tensor(
                    out=pr2.ap()[:, t % 2, kg_, :, 0:514],
                    in0=gt.ap()[:, kg_, :, 2, 0:514],
                    in1=(hcur.ap()[:, 1 + kg_:2 + kg_, 0:514]
                         .broadcast_to([128, 3, 514])),
                    op=AluOpType.mult,
                ).then_inc(s_mulp, 1)

            def pool_k6(t, hcur):
                gp.tensor_tensor(
                    out=pr2.ap()[:, t % 2, 2, 0, 0:514],
                    in0=gt.ap()[:, 2, 0, 2, 0:514],
                    in1=hcur.ap()[:, 3, 0:514],
                    op=AluOpType.mult,
                ).then_inc(s_mulp, 1)

            # setup scale-muls for taps k=6,7 of each slot
            gp.wait_ge(s_hz, 14)   # gt pad memsets (WAW on shared columns)
            for s in range(NS):
                gp.wait_ge(s_rexp, s + 1)
                for k in POOL_SETUP_KS:
                    kg_, kj_ = gk(k)
                    dj = OFFS[k][1]
                    gp.tensor_tensor(
                        out=gt.ap()[:, kg_, kj_, s, 1 + dj:513 + dj],
                        in0=graw.ap()[:, CH_FOR_K[k], s, 1:513],
                        in1=rr.ap()[:, s, 1:513],
                        op=AluOpType.mult,
                    ).then_inc(s_gtp, 1)
            # t=0: kg1 holds the center tap (needs cev2); run it last
            gp.wait_ge(s_gtd, 18)
            gp.wait_ge(s_gtp, 8)   # own setup-mul writes retired
            gp.wait_ge(s_x, 16)
            pool_kg(0, h0, 0)
            pool_k6(0, h0)
            gp.wait_ge(s_cev, 3)
            pool_kg(0, h0, 1)
            for t in range(1, niter):
                hcur = hb[t % 2]
                gp.wait_ge(s_ev, 4 * (t - 1) + EVPOS[1])
                if t >= 2:
                    gp.wait_ge(s_pe, 6 * (t - 2) + 5)
                pool_kg(t, hcur, 0)
                gp.wait_ge(s_ev, 4 * (t - 1) + EVPOS[2])
                pool_kg(t, hcur, 1)
                gp.wait_ge(s_ev, 4 * (t - 1) + EVPOS[3])
                pool_k6(t, hcur)
            gp.wait_ge(s_out, 64)

        # ---------------- DVE: memsets, abs, setup muls, products ------------
        @block.vector
        def _(v):
            v.memset(ident.ap(), 0.0).then_inc(s_hz, 1)
            v.memset(wup.ap(), 0.0).then_inc(s_hz, 1)
            v.memset(wdn.ap(), 0.0).then_inc(s_hz, 1)
            # h pad columns only; bodies are written by the x DMA (h0) and
            # by evictions (h1) before any read
            v.memset(h0.ap()[:, :, 0:1], 0.0).then_inc(s_hz, 1)
            v.memset(h0.ap()[:, :, 513:520], 0.0).then_inc(s_hz, 1)
            v.memset(c_eps.ap(), 1e-8).then_inc(s_hz, 1)
            v.memset(h1.ap()[:, :, 0:1], 0.0).then_inc(s_hz, 1)
            v.memset(h1.ap()[:, :, 513:520], 0.0).then_inc(s_hz, 1)
            # abs per slot as guidance DMAs land; pad memsets fill the gaps
            # (their consumers are transitively gated through s_gtd/s_mul)
            for s in range(NS):
                v.wait_ge(s_gs[s], 16)
                if s >= 2:
                    v.wait_ge(s_apex, s - 1)  # gabs[s%2] free again
                v.tensor_scalar(
                    out=gabs.ap()[:, s % 2, :, 1:513].bitcast(U16),
                    in0=graw.ap()[:, :, s, 1:513].bitcast(U16),
                    scalar1=0x7FFF, scalar2=None, op0=AluOpType.bitwise_and,
                ).then_inc(s_abs, 1)
                if s == 0:
                    for kg_ in range(3):
                        v.memset(gt.ap()[:, kg_, :, :, 0:2], 0.0).then_inc(s_hz, 1)
                        v.memset(gt.ap()[:, kg_, :, :, 512:520], 0.0).then_inc(s_hz, 1)
                else:
                    # slot s-1 scale-muls, interleaved so they start as soon
                    # as exp(s-1) lands instead of after all four abs passes
                    emit_dve_gtmuls(v, s - 1)
            emit_dve_gtmuls(v, 3)
            # iteration products: 7 instructions per iteration, ordered by
            # gate availability then consumer deadline.  PE window order:
            # 0a'(k678), slot1, 0a''(k345), 0b(cross), slot2, slot3.
            # t=0 uses a different order and precise setup gates (only the
            # tap-groups containing the center k=4 need the center-evict).
            def p_s0kg2(t, hcur):
                v.tensor_tensor(
                    out=pr.ap()[:, 2, :, 0, 0:514],
                    in0=gt.ap()[:, 2, :, 0, 0:514],
                    in1=(hcur.ap()[:, 1:2, 0:514]
                         .broadcast_to([128, 3, 514])),
                    op=AluOpType.mult,
                ).then_inc(s_mul, 1)

            def p_s1kg01(t, hcur):
                v.tensor_tensor(
                    out=pr.ap()[:, 0:2, :, 1, 0:514],
                    in0=gt.ap()[:, 0:2, :, 1, 0:514],
                    in1=h_bcast(hcur, 0, 2, 128), op=AluOpType.mult,
                ).then_inc(s_mul, 1)

            def p_s1kg2(t, hcur):
                v.tensor_tensor(
                    out=pr.ap()[:, 2, :, 1, 0:514],
                    in0=gt.ap()[:, 2, :, 1, 0:514],
                    in1=(hcur.ap()[:, 2:3, 0:514]
                         .broadcast_to([128, 3, 514])),
                    op=AluOpType.mult,
                ).then_inc(s_mul, 1)

            def p_s0kg1(t, hcur):
                v.tensor_tensor(
                    out=pr.ap()[:, 1, :, 0, 0:514],
                    in0=gt.ap()[:, 1, :, 0, 0:514],
                    in1=(hcur.ap()[:, 0:1, 0:514]
                         .broadcast_to([128, 3, 514])),
                    op=AluOpType.mult,
                ).then_inc(s_mul, 1)

            def p_up3(t, hcur):
                v.tensor_tensor(
                    out=prc.ap()[:, 0:3, 0:514],
                    in0=gtc.ap()[:, 0:3, 0:514],
                    in1=(hcur.ap()[:, 3:4, 0:514]
                         .broadcast_to([128, 3, 514])),
                    op=AluOpType.mult,
                ).then_inc(s_mul, 1)

            def p_s3kg01(t, hcur):
                v.tensor_tensor(
                    out=pr.ap()[:, 0:2, :, 3, 0:514],
                    in0=gt.ap()[:, 0:2, :, 3, 0:514],
                    in1=h_bcast(hcur, 2, 4, 128), op=AluOpType.mult,
                ).then_inc(s_mul, 1)

            def p_s2k78(t, hcur):
                v.tensor_tensor(
                    out=pr2.ap()[:, t % 2, 2, 1:3, 0:514],
                    in0=gt.ap()[:, 2, 1:3, 2, 0:514],
                    in1=(hcur.ap()[:, 3:4, 0:514]
                         .broadcast_to([128, 2, 514])),
                    op=AluOpType.mult,
                ).then_inc(s_mul, 1)

            def p_dn3(t, hcur):
                v.tensor_tensor(
                    out=prc.ap()[:, 3:6, 0:514],
                    in0=gtc.ap()[:, 3:6, 0:514],
                    in1=(hcur.ap()[:, 0:1, 0:514]
                         .broadcast_to([128, 3, 514])),
                    op=AluOpType.mult,
                ).then_inc(s_mul, 1)

            # t=0: split the center-bearing tap-groups so only three tail
            # instructions wait on center-evicts; everything else is gated by
            # the scale-muls / gtc shifts alone
            def p_kg(sg, kg_, hslot):
                v.tensor_tensor(
                    out=pr.ap()[:, kg_, :, sg, 0:514],
                    in0=gt.ap()[:, kg_, :, sg, 0:514],
                    in1=(h0.ap()[:, hslot:hslot + 1, 0:514]
                         .broadcast_to([128, 3, 514])),
                    op=AluOpType.mult,
                ).then_inc(s_mul, 1)

            v.wait_ge(s_gtd, 6)
            v.wait_ge(s_gtp, 2)
            v.wait_ge(s_x, 16)
            p_s0kg2(0, h0)            # 1
            v.wait_ge(s_gtd, 12)
            v.wait_ge(s_gtp, 4)
            p_kg(1, 0, 0)             # 2: slot1 k0-2
            p_s1kg2(0, h0)            # 3
            v.wait_ge(s_gtcu, 32)
            p_up3(0, h0)              # 4
            v.wait_ge(s_gtcd, 32)
            p_dn3(0, h0)              # 5
            v.wait_ge(s_gtd, 24)
            p_kg(3, 0, 2)             # 6: slot3 k0-2
            v.wait_ge(s_gtp, 6)
            p_s2k78(0, h0)            # 7: slot2 k7,k8
            v.wait_ge(s_cev, 1)
            p_kg(0, 1, 0)             # 8: slot0 k3-5 (center)
            v.wait_ge(s_cev, 2)
            p_kg(1, 1, 1)             # 9: slot1 k3-5 (center)
            v.wait_ge(s_cev, 4)
            p_kg(3, 1, 3)             # 10: slot3 k3-5 (center)
            # t>=1: steady-state order and gates
            for t in range(1, niter):
                hcur = hb[t % 2]
                v.wait_ge(s_ev, 4 * (t - 1) + 1)
                v.wait_ge(s_pe, 6 * (t - 1) + 1)
                p_s0kg2(t, hcur)
                v.wait_ge(s_ev, 4 * (t - 1) + 2)
                v.wait_ge(s_pe, 6 * (t - 1) + 2)
                p_s1kg01(t, hcur)
                v.wait_ge(s_ev, 4 * (t - 1) + 3)
                p_s1kg2(t, hcur)
                v.wait_ge(s_pe, 6 * (t - 1) + 3)
                p_s0kg1(t, hcur)
                v.wait_ge(s_ev, 4 * (t - 1) + 4)
                v.wait_ge(s_pe, 6 * (t - 1) + 4)
                p_up3(t, hcur)
                if t >= 2:
                    v.wait_ge(s_pe, 6 * (t - 2) + 5)   # pr2 parity WAR
                p_s2k78(t, hcur)
                v.wait_ge(s_pe, 6 * (t - 1) + 6)
                p_s3kg01(t, hcur)
                p_dn3(t, hcur)

        # ---------------- PE: setup sums + iteration tap-sums ----------------
        @block.tensor
        def _(pe):
            pe.wait_ge(s_id, 3)
            # warm-up: ramp the PE p-state past the 3us threshold on dummy
            # 128-col matmuls so the first A-sum runs at full clock
            for _ in range(30):
                pe.matmul(pg(1, 0)[:, 0:128], ident.ap(), ident.ap(),
                          start=True, stop=True, skip_group_check=True)
            # A = sum_ch |g_ch| into psum group 0
            for s in range(NS):
                pe.wait_ge(s_abs, s + 1)
                for ch in range(8):
                    inst = pe.matmul(
                        pg(0, s)[:, 0:512], ident.ap(),
                        gabs.ap()[:, s % 2, ch, 1:513],
                        start=(ch == 0), stop=(ch == 7),
                    )
                    if ch == 7:
                        inst.then_inc(s_apex, 1)
            # center gate: sum of fp16-rounded weights into psum group 1
            for s in range(NS):
                pe.wait_ge(s_gtd, len(DVE_SETUP_KS) * (s + 1))
                pe.wait_ge(s_gtp, len(POOL_SETUP_KS) * (s + 1))
                done = 0
                for k in range(9):
                    if k == 4:
                        continue
                    kg_, kj_ = gk(k)
                    dj = OFFS[k][1]
                    inst = pe.matmul(
                        pg(1, s)[:, 0:512], ident.ap(),
                        gt.ap()[:, kg_, kj_, s, 1 + dj:513 + dj],
                        start=(done == 0), stop=(done == 7),
                    )
                    done += 1
                    if done == 8:
                        inst.then_inc(s_cpe, 1)
            # keepalive: bridge the idle gap between setup and the first
            # window so the p-state ramp survives into T0's matmuls.
            # pg(0,0)'s only setup reader (ln0) is gated done via s_ln.
            pe.wait_ge(s_ln, 1)
            for _ in range(N_KEEPALIVE):
                pe.matmul(pg(0, 0)[:, 0:128], ident.ap(), ident.ap(),
                          start=True, stop=True, skip_group_check=True)
            # iterations: window order 0a'(k678, start psum0), slot1,
            # 0a''(k345), 0b(cross, stop psum0), slot2, slot3;
            # s_pe +1 after each of the 6 groups
            for t in range(niter):
                # t=0 DVE emission order differs; map each group to the
                # s_mul count of its last-needed product
                mw = ([1, 9, 8, 4, 7, 10] if t == 0
                      else [N_T0_MUL + 8 * (t - 1) + 1, N_T0_MUL + 8 * (t - 1) + 3,
                            N_T0_MUL + 8 * (t - 1) + 4, N_T0_MUL + 8 * (t - 1) + 5,
                            N_T0_MUL + 8 * (t - 1) + 6, N_T0_MUL + 8 * (t - 1) + 8])
                groups = [
                    # (psum slot, taps, s_mul wait, s_mulp wait, start, stop)
                    (0, [(k, ident, None) for k in range(6, 9)],
                     mw[0], None, True, False),
                    (1, [(k, ident, None) for k in range(9)],
                     mw[1], None, True, True),
                    (0, [(k, ident, None) for k in range(3, 6)],
                     mw[2], None, False, False),
                    (0, [(k, wup, k) for k in range(3)],
                     mw[3], None, False, True),
                    (2, [(k, ident, None) for k in range(9)],
                     mw[4], 3 * t + 3, True, True),
                    (3, [(k, ident, None) for k in range(6)]
                        + [(k, wdn, 3 + k - 6) for k in range(6, 9)],
                     mw[5], None, True, True),
                ]
                for sg, taps, mw, mpw, st, sp in groups:
                    if mw is not None:
                        pe.wait_ge(s_mul, mw)
                    if mpw is not None:
                        pe.wait_ge(s_mulp, mpw)
                    if st:
                        # psum(sg, t%2) must be drained by its previous reader
                        if t == 0:
                            pe.wait_ge(s_ln, sg + 1)
                        elif t == 1:
                            pe.wait_ge(s_cev, sg + 1)
                        else:
                            pe.wait_ge(s_ev, 4 * (t - 2) + EVPOS[sg])
                    n = len(taps)
                    for j, (k, w_t, crow) in enumerate(taps):
                        kg_, kj_ = gk(k)
                        dj = OFFS[k][1]
                        if sg == 2:
                            mv = pr2.ap()[:, t % 2, kg_, kj_, 1 + dj:513 + dj]
                        elif crow is None:
                            mv = pr.ap()[:, kg_, kj_, sg, 1 + dj:513 + dj]
                        else:
                            mv = prc.ap()[:, crow, 1 + dj:513 + dj]
                        inst = pe.matmul(
                            pg(t % 2, sg)[:, 0:512], w_t.ap(), mv,
                            start=(st and j == 0), stop=(sp and j == n - 1),
                            skip_group_check=True,
                        )
                        if j == n - 1:
                            inst.then_inc(s_pe, 1)

        # ---------------- ACT: ln/exp normalization, evictions ---------------
        @block.scalar
        def _(sc):
            sc.wait_ge(s_hz, 6)   # c_eps ready
            for s in range(NS):
                sc.wait_ge(s_apex, s + 1)
                sc.activation(
                    asb.ap()[:, s, 1:513], pg(0, s)[:, 0:512], AF.Ln,
                    bias=c_eps.ap(),
                ).then_inc(s_ln, 1)
                sc.wait_ge(s_ln, s + 1)
                sc.activation(
                    rr.ap()[:, s, 1:513], asb.ap()[:, s, 1:513], AF.Exp,
                    scale=-1.0,
                ).then_inc(s_rexp, 1)
            for s in range(NS):
                sc.wait_ge(s_cpe, s + 1)
                sc.activation(
                    gt.ap()[:, 1, 1, s, 1:513], pg(1, s)[:, 0:512], AF.Identity,
                    bias=1.0, scale=-1.0,
                ).then_inc(s_cev, 1)
            # eviction order [1, 0, 2, 3] matches PE stop-group order
            pe_stop = {1: 2, 0: 4, 2: 5, 3: 6}
            # DVE cycle-(t-1) readers of h slot sg (last reading instr index)
            war_mul = {0: 8, 1: 2, 2: 7, 3: 7}
            war_mulp = {1: 1, 2: 2, 3: 3}
            for t in range(niter):
                hnext = hb[(t + 1) % 2]
                last = (t == niter - 1)
                for sg in EVORD:
                    sc.wait_ge(s_pe, 6 * t + pe_stop[sg])
                    if t == 0 and sg == EVORD[0]:
                        sc.wait_ge(s_hz, 8)   # h1 pad memsets done
                    if t > 0:
                        # h_next WAR: cycle t-1 readers of this buffer+slot
                        sc.wait_ge(s_mul, N_T0_MUL if t == 1
                                   else N_T0_MUL + 8 * (t - 2) + war_mul[sg])
                        if sg in war_mulp:
                            sc.wait_ge(s_mulp, 3 * (t - 1) + war_mulp[sg])
                    if last:
                        # final eviction straight to fp32 at true magnitude
                        # (undoes the 23 earlier rescales plus this one)
                        sc.activation(
                            asb.ap()[:, sg, 1:513],
                            pg(t % 2, sg)[:, 0:512], AF.Copy,
                            scale=RESCALE * float(1.0 / RESCALE) ** niter,
                        ).then_inc(s_fin, 1)
                    else:
                        sc.activation(
                            hnext.ap()[:, sg, 1:513],
                            pg(t % 2, sg)[:, 0:512], AF.Copy,
                            scale=RESCALE,
                        ).then_inc(s_ev, 1)

        # ---------------- SYNC: gtc shift DMAs + output ----------------------
        @block.sync
        def _(sy):
            # cross weights: one-time partition shifts of the normalized gt
            sy.wait_ge(s_gtd, 3)
            sy.dma_start(gtc.ap()[0:127, 0:3, 0:514],
                         gt.ap()[1:128, 0, :, 0, 0:514]).then_inc(s_gtcu, 16)
            sy.dma_start(gtc.ap()[127:128, 0:3, 0:514],
                         gt.ap()[0:1, 0, :, 0, 0:514]).then_inc(s_gtcu, 16)
            sy.wait_ge(s_gtd, len(DVE_SETUP_KS) * NS)
            sy.wait_ge(s_gtp, len(POOL_SETUP_KS) * NS)
            sy.dma_start(gtc.ap()[1:128, 3:6, 0:514],
                         gt.ap()[0:127, 2, :, 3, 0:514]).then_inc(s_gtcd, 16)
            sy.dma_start(gtc.ap()[0:1, 3:6, 0:514],
                         gt.ap()[127:128, 2, :, 3, 0:514]).then_inc(s_gtcd, 16)
            # per-slot output DMAs, each as soon as its final eviction lands
            for i, sg in enumerate(EVORD):
                sy.wait_ge(s_fin, i + 1)
                o_out = bass.AP(o_dram, sg * W, [[4 * W, 128], [1, W]])
                sy.dma_start(o_out, asb.ap()[:, sg, 1:513]).then_inc(s_out, 16)

    return nc


_NC_CACHE = {}


def kernel(guidance: np.ndarray, x: np.ndarray) -> np.ndarray:
    """guidance [8,8,512,512] f32, x [8,1,512,512] f32 -> [8,1,512,512] f32."""
    guidance = np.ascontiguousarray(np.asarray(guidance, dtype=np.float32))
    x = np.ascontiguousarray(np.asarray(x, dtype=np.float32))
    if "nc" not in _NC_CACHE:
        _NC_CACHE["nc"] = build_program()
    nc = _NC_CACHE["nc"]
    in_maps = [
        {"guidance": guidance[b], "x": x[b].reshape(1, H, W)}
        for b in range(N_CORES)
    ]
    res = run_bass_kernel_spmd(nc, in_maps, core_ids=list(range(N_CORES)))
    out = np.stack([res.results[b]["out"] for b in range(N_CORES)], axis=0)
    return out.astype(np.float32)


# revision 6
# speedup vs baseline: 1.0137x; 1.0060x over previous
"""CSPN (convolutional spatial propagation) Trainium2 kernel, v2.

Full inputs:  guidance [8, 8, 512, 512] f32, x [8, 1, 512, 512] f32.
Sharding: data-parallel over batch -- core b gets batch element b.

v2 redesign vs the halo-DMA baseline:
  * No halo rows at all.  h is [128, 4, PW]: partition p holds image rows
    4p..4p+3 (slots 0-3).  The 6 tap-rows whose input row lives on a
    neighbouring partition (slot0 di=-1, slot3 di=+1) are multiplied ON the
    partition that owns the h row, against one-time partition-shifted copies
    of the weights (gtc).  The PE re-aligns those products to the output
    partition with shifted-identity stationary matrices (free in both the
    cost model and -- via dual weight buffers -- nearly free on HW).
  * Product rows are split DVE 29 / Pool 7 per iteration (the real
    TRN2 ISA allows only plain tensor_tensor on Pool -- the faster
    TensorScalarPtr family is DVE-only).  Setup scale-muls all run
    on DVE in 2x mode (rr stored f16); PE p-state is kept warm with
    dummy-matmul keepalive blocks.
  * DVE products are merged into 5 wide instructions per iteration
    ([2-3, 3, 514] APs with double-broadcast h), cutting per-instruction
    overhead; Pool does one [3, 3, 514] instruction (slot 2).
  * Steady state is PE-bound at 36 matmul-columns x 512 per iteration
    (~7.7 us); DVE ~7.5 us, Pool ~6.5 us, ACT ~2.5 us.

Numerics are identical to the baseline: fp16 weights/products, fp32 PSUM
accumulation, center gate gt4 = 1 - sum(fp16-rounded gt_k), h rescaled by
0.5/iter to stay in fp16 range, un-scaled by 2^24 in the final fp32 pass.
"""

import sys

sys.path.insert(0, "/opt/trn_rl_repo")

import numpy as np

import concourse.bass as bass
from concourse import mybir
from concourse.bass_utils import run_bass_kernel_spmd
from concourse.alu_op_type import AluOpType

F16 = mybir.dt.float16
F32 = mybir.dt.float32
U16 = mybir.dt.uint16
AF = mybir.ActivationFunctionType

N_CORES = 8
H, W = 512, 512
NS = 4            # row-slots per partition
PW = 520          # f16 elements per row slot (514 used + pad)
NITER = 24
RESCALE = 0.5
# tap k = (di+1)*3 + (dj+1); k=4 is the center gate
OFFS = [(k // 3 - 1, k % 3 - 1) for k in range(9)]
# guidance channel for tap k (center has none)
CH_FOR_K = [0, 1, 2, 3, None, 4, 5, 6, 7]
# setup scale-muls: 6 on DVE (rr f16 -> 2x mode), 2 on the otherwise-idle
# Pool via plain tensor_tensor (the only mult op the real ISA allows there)
DVE_SETUP_KS = (0, 1, 2, 3, 5, 8)
POOL_SETUP_KS = (6, 7)
# eviction order within a cycle (follows the PE stop-group order) and the
# 1-based position of each slot's eviction in that order
EVORD = [1, 0, 2, 3]
EVPOS = {1: 1, 0: 2, 2: 3, 3: 4}
# DVE product-instruction count at t=0 (9: split center groups)
N_T0_MUL = 10
N_KEEPALIVE = 60
# column split of the slot-2 k7/k8 rows between Pool [0:CSPL] and DVE
CSPL = 57


def build_program(niter=NITER):
    nc = bass.Bass("TRN2", target_bir_lowering=False, debug=False)

    g_dram = nc.dram_tensor("guidance", [8, H, W], F32, kind="ExternalInput")
    x_dram = nc.dram_tensor("x", [1, H, W], F32, kind="ExternalInput")
    o_dram = nc.dram_tensor("out", [1, H, W], F32, kind="ExternalOutput")

    h0 = nc.alloc_sbuf_tensor("h0", [128, NS, PW], F16)
    h1 = nc.alloc_sbuf_tensor("h1", [128, NS, PW], F16)
    # weights / products indexed [kg, kj, slot, col] with tap k = 3*kg + kj
    gt = nc.alloc_sbuf_tensor("gt", [128, 3, 3, NS, PW], F16)
    pr = nc.alloc_sbuf_tensor("pr", [128, 3, 3, NS, PW], F16)
    # slot-2 products, double-buffered so Pool(t+1) never waits on PE2(t)
    pr2 = nc.alloc_sbuf_tensor("pr2", [128, 2, 3, 3, PW], F16)
    # cross rows: 0-2 = slot0 taps k=0,1,2 shifted up (gtc[p] = gt[p+1]);
    #             3-5 = slot3 taps k=6,7,8 shifted down (gtc[p] = gt[p-1])
    gtc = nc.alloc_sbuf_tensor("gtc", [128, 6, PW], F16)
    prc = nc.alloc_sbuf_tensor("prc", [128, 6, PW], F16)
    graw = nc.alloc_sbuf_tensor("graw", [128, 8, NS, PW], F16)
    gabs = nc.alloc_sbuf_tensor("gabs", [128, 2, 8, PW], F16)
    asb = nc.alloc_sbuf_tensor("asb", [128, NS, PW], F32)   # ln(A); out stage
    rr = nc.alloc_sbuf_tensor("rr", [128, NS, PW], F16)     # r = 1/A
    ident = nc.alloc_sbuf_tensor("ident", [128, 128], F16)
    wup = nc.alloc_sbuf_tensor("wup", [128, 128], F16)      # psum[p] += x[p-1]
    wdn = nc.alloc_sbuf_tensor("wdn", [128, 128], F16)      # psum[p] += x[p+1]
    c_eps = nc.alloc_sbuf_tensor("c_eps", [128, 1], F32)

    psum = [nc.alloc_psum_tensor(f"pg{g}s{s}", [128, W], F32)
            for g in range(2) for s in range(NS)]

    def pg(g, s):
        return psum[g * NS + s].ap()

    hb = [h0, h1]

    s_hz = nc.alloc_semaphore("s_hz")      # DVE memsets done
    s_id = nc.alloc_semaphore("s_id")      # stationaries built (3)
    s_x = nc.alloc_semaphore("s_x")        # x DMA (+16)
    s_gs = [nc.alloc_semaphore(f"s_g{i}") for i in range(NS)]
    s_abs = nc.alloc_semaphore("s_abs")    # DVE abs per slot
    s_apex = nc.alloc_semaphore("s_apex")  # PE A-sum per slot
    s_ln = nc.alloc_semaphore("s_ln")      # ACT ln per slot
    s_rexp = nc.alloc_semaphore("s_rexp")  # ACT exp per slot
    s_gtd = nc.alloc_semaphore("s_gtd")    # DVE setup muls (5/slot)
    s_gtp = nc.alloc_semaphore("s_gtp")    # Pool setup muls (3/slot)
    s_cpe = nc.alloc_semaphore("s_cpe")    # PE center sum per slot
    s_cev = nc.alloc_semaphore("s_cev")    # ACT center evict per slot
    s_gtcu = nc.alloc_semaphore("s_gtcu")  # gtc up-shift DMA (+16)
    s_gtcd = nc.alloc_semaphore("s_gtcd")  # gtc down-shift DMA (+16)
    s_mul = nc.alloc_semaphore("s_mul")    # DVE iter muls (+5/iter)
    s_mulp = nc.alloc_semaphore("s_mulp")  # Pool iter muls (+1/iter)
    s_pe = nc.alloc_semaphore("s_pe")      # PE tap-sum per slot (+4/iter)
    s_ev = nc.alloc_semaphore("s_ev")      # ACT evict per slot (+4/iter)
    s_fin = nc.alloc_semaphore("s_fin")    # final ACT rescale (4)
    s_out = nc.alloc_semaphore("s_out")    # output DMA

    N_MEMSET = 6

    def gk(k):
        return k // 3, k % 3

    def emit_dve_gtmuls(v, s):
        v.wait_ge(s_hz, 14)   # gt pad memsets retired (same-engine WAW)
        v.wait_ge(s_rexp, s + 1)
        for k in DVE_SETUP_KS:
            kg_, kj_ = gk(k)
            dj = OFFS[k][1]
            v.tensor_tensor(
                out=gt.ap()[:, kg_, kj_, s, 1 + dj:513 + dj],
                in0=graw.ap()[:, CH_FOR_K[k], s, 1:513],
                in1=rr.ap()[:, s, 1:513],
                op=AluOpType.mult,
            ).then_inc(s_gtd, 1)

    def h_bcast(h, lo, hi, width):
        """h slots [lo:hi] broadcast over the 3-tap (kj) dim."""
        n = hi - lo
        return (h.ap()[:, lo:hi, 0:514]
                .unsqueeze(2).broadcast_to([width, n, 3, 514]))

    with nc.Block() as block:

        # ---------------- GPSIMD/Pool: DMAs, stationaries, products ----------
        @block.gpsimd
        def _(gp):
            # guidance slot 0 first -- needs no memsets, so it starts at t=0
            g_in0 = bass.AP(g_dram, 0, [[4 * W, 128], [H * W, 8], [1, W]])
            gp.dma_start(graw.ap()[:, :, 0, 1:513], g_in0).then_inc(s_gs[0], 16)
            gp.wait_ge(s_hz, 3)
            for w_t, base in ((ident, 0), (wup, 1), (wdn, -1)):
                gp.affine_select(
                    out=w_t.ap(), in_=w_t.ap(),
                    compare_op=AluOpType.not_equal, fill=1.0, base=base,
                    pattern=[[-1, 128]], channel_multiplier=1,
                ).then_inc(s_id, 1)
            for s in range(1, NS):
                g_in = bass.AP(g_dram, s * W,
                               [[4 * W, 128], [H * W, 8], [1, W]])
                gp.dma_start(graw.ap()[:, :, s, 1:513], g_in).then_inc(s_gs[s], 16)
            # x last: it is not needed until the first products (~cev0 time)
            gp.wait_ge(s_hz, 5)
            x_in = bass.AP(x_dram, 0, [[4 * W, 128], [W, NS], [1, W]])
            gp.dma_start(h0.ap()[:, :, 1:513], x_in).then_inc(s_x, 16)
            # iteration products: slot 2, one stt per tap-triple (kg) so each
            # starts as soon as its h slot (1+kg) is evicted
            def pool_kg(t, hcur, kg_):
                gp.tensor_tensor(
                    out=pr2.ap()[:, t % 2, kg_, :, 0:514],
                    in0=gt.ap()[:, kg_, :, 2, 0:514],
                    in1=(hcur.ap()[:, 1 + kg_:2 + kg_, 0:514]
                         .broadcast_to([128, 3, 514])),
                    op=AluOpType.mult,
                ).then_inc(s_mulp, 1)

            def pool_k6(t, hcur):
                gp.tensor_tensor(
                    out=pr2.ap()[:, t % 2, 2, 0, 0:514],
                    in0=gt.ap()[:, 2, 0, 2, 0:514],
                    in1=hcur.ap()[:, 3, 0:514],
                    op=AluOpType.mult,
                ).then_inc(s_mulp, 1)

            def pool_k78s(t, hcur):
                gp.tensor_tensor(
                    out=pr2.ap()[:, t % 2, 2, 1:3, 0:CSPL],
                    in0=gt.ap()[:, 2, 1:3, 2, 0:CSPL],
                    in1=(hcur.ap()[:, 3:4, 0:CSPL]
                         .broadcast_to([128, 2, CSPL])),
                    op=AluOpType.mult,
                ).then_inc(s_mulp, 1)

            # setup scale-muls for taps k=6,7 of each slot
            gp.wait_ge(s_hz, 14)   # gt pad memsets (WAW on shared columns)
            for s in range(NS):
                gp.wait_ge(s_rexp, s + 1)
                for k in POOL_SETUP_KS:
                    kg_, kj_ = gk(k)
                    dj = OFFS[k][1]
                    gp.tensor_tensor(
                        out=gt.ap()[:, kg_, kj_, s, 1 + dj:513 + dj],
                        in0=graw.ap()[:, CH_FOR_K[k], s, 1:513],
                        in1=rr.ap()[:, s, 1:513],
                        op=AluOpType.mult,
                    ).then_inc(s_gtp, 1)
            # t=0: kg1 holds the center tap (needs cev2); run it last
            gp.wait_ge(s_gtd, 18)
            gp.wait_ge(s_gtp, 8)   # own setup-mul writes retired
            gp.wait_ge(s_x, 16)
            pool_kg(0, h0, 0)
            pool_k6(0, h0)
            pool_k78s(0, h0)
            gp.wait_ge(s_cev, 3)
            pool_kg(0, h0, 1)
            for t in range(1, niter):
                hcur = hb[t % 2]
                gp.wait_ge(s_ev, 4 * (t - 1) + EVPOS[1])
                if t >= 2:
                    gp.wait_ge(s_pe, 6 * (t - 2) + 5)
                pool_kg(t, hcur, 0)
                gp.wait_ge(s_ev, 4 * (t - 1) + EVPOS[2])
                pool_kg(t, hcur, 1)
                gp.wait_ge(s_ev, 4 * (t - 1) + EVPOS[3])
                pool_k6(t, hcur)
                pool_k78s(t, hcur)
            gp.wait_ge(s_out, 64)

        # ---------------- DVE: memsets, abs, setup muls, products ------------
        @block.vector
        def _(v):
            v.memset(ident.ap(), 0.0).then_inc(s_hz, 1)
            v.memset(wup.ap(), 0.0).then_inc(s_hz, 1)
            v.memset(wdn.ap(), 0.0).then_inc(s_hz, 1)
            # h pad columns only; bodies are written by the x DMA (h0) and
            # by evictions (h1) before any read
            v.memset(h0.ap()[:, :, 0:1], 0.0).then_inc(s_hz, 1)
            v.memset(h0.ap()[:, :, 513:520], 0.0).then_inc(s_hz, 1)
            v.memset(c_eps.ap(), 1e-8).then_inc(s_hz, 1)
            v.memset(h1.ap()[:, :, 0:1], 0.0).then_inc(s_hz, 1)
            v.memset(h1.ap()[:, :, 513:520], 0.0).then_inc(s_hz, 1)
            # abs per slot as guidance DMAs land; pad memsets fill the gaps
            # (their consumers are transitively gated through s_gtd/s_mul)
            for s in range(NS):
                v.wait_ge(s_gs[s], 16)
                if s >= 2:
                    v.wait_ge(s_apex, s - 1)  # gabs[s%2] free again
                v.tensor_scalar(
                    out=gabs.ap()[:, s % 2, :, 1:513].bitcast(U16),
                    in0=graw.ap()[:, :, s, 1:513].bitcast(U16),
                    scalar1=0x7FFF, scalar2=None, op0=AluOpType.bitwise_and,
                ).then_inc(s_abs, 1)
                if s == 0:
                    for kg_ in range(3):
                        v.memset(gt.ap()[:, kg_, :, :, 0:2], 0.0).then_inc(s_hz, 1)
                        v.memset(gt.ap()[:, kg_, :, :, 512:520], 0.0).then_inc(s_hz, 1)
                else:
                    # slot s-1 scale-muls, interleaved so they start as soon
                    # as exp(s-1) lands instead of after all four abs passes
                    emit_dve_gtmuls(v, s - 1)
            emit_dve_gtmuls(v, 3)
            # iteration products: 7 instructions per iteration, ordered by
            # gate availability then consumer deadline.  PE window order:
            # 0a'(k678), slot1, 0a''(k345), 0b(cross), slot2, slot3.
            # t=0 uses a different order and precise setup gates (only the
            # tap-groups containing the center k=4 need the center-evict).
            def p_s0kg2(t, hcur):
                v.tensor_tensor(
                    out=pr.ap()[:, 2, :, 0, 0:514],
                    in0=gt.ap()[:, 2, :, 0, 0:514],
                    in1=(hcur.ap()[:, 1:2, 0:514]
                         .broadcast_to([128, 3, 514])),
                    op=AluOpType.mult,
                ).then_inc(s_mul, 1)

            def p_s1kg01(t, hcur):
                v.tensor_tensor(
                    out=pr.ap()[:, 0:2, :, 1, 0:514],
                    in0=gt.ap()[:, 0:2, :, 1, 0:514],
                    in1=h_bcast(hcur, 0, 2, 128), op=AluOpType.mult,
                ).then_inc(s_mul, 1)

            def p_s1kg2(t, hcur):
                v.tensor_tensor(
                    out=pr.ap()[:, 2, :, 1, 0:514],
                    in0=gt.ap()[:, 2, :, 1, 0:514],
                    in1=(hcur.ap()[:, 2:3, 0:514]
                         .broadcast_to([128, 3, 514])),
                    op=AluOpType.mult,
                ).then_inc(s_mul, 1)

            def p_s0kg1(t, hcur):
                v.tensor_tensor(
                    out=pr.ap()[:, 1, :, 0, 0:514],
                    in0=gt.ap()[:, 1, :, 0, 0:514],
                    in1=(hcur.ap()[:, 0:1, 0:514]
                         .broadcast_to([128, 3, 514])),
                    op=AluOpType.mult,
                ).then_inc(s_mul, 1)

            def p_up3(t, hcur):
                v.tensor_tensor(
                    out=prc.ap()[:, 0:3, 0:514],
                    in0=gtc.ap()[:, 0:3, 0:514],
                    in1=(hcur.ap()[:, 3:4, 0:514]
                         .broadcast_to([128, 3, 514])),
                    op=AluOpType.mult,
                ).then_inc(s_mul, 1)

            def p_s3kg01(t, hcur):
                v.tensor_tensor(
                    out=pr.ap()[:, 0:2, :, 3, 0:514],
                    in0=gt.ap()[:, 0:2, :, 3, 0:514],
                    in1=h_bcast(hcur, 2, 4, 128), op=AluOpType.mult,
                ).then_inc(s_mul, 1)

            def p_s2k78(t, hcur):
                # cols [CSPL:514]; Pool covers [0:CSPL] of the same rows
                v.tensor_tensor(
                    out=pr2.ap()[:, t % 2, 2, 1:3, CSPL:514],
                    in0=gt.ap()[:, 2, 1:3, 2, CSPL:514],
                    in1=(hcur.ap()[:, 3:4, CSPL:514]
                         .broadcast_to([128, 2, 514 - CSPL])),
                    op=AluOpType.mult,
                ).then_inc(s_mul, 1)

            def p_dn3(t, hcur):
                v.tensor_tensor(
                    out=prc.ap()[:, 3:6, 0:514],
                    in0=gtc.ap()[:, 3:6, 0:514],
                    in1=(hcur.ap()[:, 0:1, 0:514]
                         .broadcast_to([128, 3, 514])),
                    op=AluOpType.mult,
                ).then_inc(s_mul, 1)

            # t=0: split the center-bearing tap-groups so only three tail
            # instructions wait on center-evicts; everything else is gated by
            # the scale-muls / gtc shifts alone
            def p_kg(sg, kg_, hslot):
                v.tensor_tensor(
                    out=pr.ap()[:, kg_, :, sg, 0:514],
                    in0=gt.ap()[:, kg_, :, sg, 0:514],
                    in1=(h0.ap()[:, hslot:hslot + 1, 0:514]
                         .broadcast_to([128, 3, 514])),
                    op=AluOpType.mult,
                ).then_inc(s_mul, 1)

            v.wait_ge(s_gtd, 6)
            v.wait_ge(s_gtp, 2)
            v.wait_ge(s_x, 16)
            p_s0kg2(0, h0)            # 1
            v.wait_ge(s_gtd, 12)
            v.wait_ge(s_gtp, 4)
            p_kg(1, 0, 0)             # 2: slot1 k0-2
            p_s1kg2(0, h0)            # 3
            v.wait_ge(s_gtcu, 32)
            p_up3(0, h0)              # 4
            v.wait_ge(s_gtcd, 32)
            p_dn3(0, h0)              # 5
            v.wait_ge(s_gtd, 24)
            p_kg(3, 0, 2)             # 6: slot3 k0-2
            v.wait_ge(s_gtp, 6)
            p_s2k78(0, h0)            # 7: slot2 k7,k8
            v.wait_ge(s_cev, 1)
            p_kg(0, 1, 0)             # 8: slot0 k3-5 (center)
            v.wait_ge(s_cev, 2)
            p_kg(1, 1, 1)             # 9: slot1 k3-5 (center)
            v.wait_ge(s_cev, 4)
            p_kg(3, 1, 3)             # 10: slot3 k3-5 (center)
            # t>=1: steady-state order and gates
            for t in range(1, niter):
                hcur = hb[t % 2]
                v.wait_ge(s_ev, 4 * (t - 1) + 1)
                v.wait_ge(s_pe, 6 * (t - 1) + 1)
                p_s0kg2(t, hcur)
                v.wait_ge(s_ev, 4 * (t - 1) + 2)
                v.wait_ge(s_pe, 6 * (t - 1) + 2)
                p_s1kg01(t, hcur)
                v.wait_ge(s_ev, 4 * (t - 1) + 3)
                p_s1kg2(t, hcur)
                v.wait_ge(s_pe, 6 * (t - 1) + 3)
                p_s0kg1(t, hcur)
                v.wait_ge(s_ev, 4 * (t - 1) + 4)
                v.wait_ge(s_pe, 6 * (t - 1) + 4)
                p_up3(t, hcur)
                if t >= 2:
                    v.wait_ge(s_pe, 6 * (t - 2) + 5)   # pr2 parity WAR
                p_s2k78(t, hcur)
                v.wait_ge(s_pe, 6 * (t - 1) + 6)
                p_s3kg01(t, hcur)
                p_dn3(t, hcur)

        # ---------------- PE: setup sums + iteration tap-sums ----------------
        @block.tensor
        def _(pe):
            pe.wait_ge(s_id, 3)
            # warm-up: ramp the PE p-state past the 3us threshold on dummy
            # 128-col matmuls so the first A-sum runs at full clock
            for _ in range(30):
                pe.matmul(pg(1, 0)[:, 0:128], ident.ap(), ident.ap(),
                          start=True, stop=True, skip_group_check=True)
            # A = sum_ch |g_ch| into psum group 0
            for s in range(NS):
                pe.wait_ge(s_abs, s + 1)
                for ch in range(8):
                    inst = pe.matmul(
                        pg(0, s)[:, 0:512], ident.ap(),
                        gabs.ap()[:, s % 2, ch, 1:513],
                        start=(ch == 0), stop=(ch == 7),
                    )
                    if ch == 7:
                        inst.then_inc(s_apex, 1)
            # center gate: sum of fp16-rounded weights into psum group 1
            for s in range(NS):
                pe.wait_ge(s_gtd, len(DVE_SETUP_KS) * (s + 1))
                pe.wait_ge(s_gtp, len(POOL_SETUP_KS) * (s + 1))
                done = 0
                for k in range(9):
                    if k == 4:
                        continue
                    kg_, kj_ = gk(k)
                    dj = OFFS[k][1]
                    inst = pe.matmul(
                        pg(1, s)[:, 0:512], ident.ap(),
                        gt.ap()[:, kg_, kj_, s, 1 + dj:513 + dj],
                        start=(done == 0), stop=(done == 7),
                    )
                    done += 1
                    if done == 8:
                        inst.then_inc(s_cpe, 1)
            # keepalive: bridge the idle gap between setup and the first
            # window so the p-state ramp survives into T0's matmuls.
            # pg(0,0)'s only setup reader (ln0) is gated done via s_ln.
            pe.wait_ge(s_ln, 1)
            for _ in range(N_KEEPALIVE):
                pe.matmul(pg(0, 0)[:, 0:128], ident.ap(), ident.ap(),
                          start=True, stop=True, skip_group_check=True)
            # iterations: window order 0a'(k678, start psum0), slot1,
            # 0a''(k345), 0b(cross, stop psum0), slot2, slot3;
            # s_pe +1 after each of the 6 groups
            for t in range(niter):
                # t=0 DVE emission order differs; map each group to the
                # s_mul count of its last-needed product
                mw = ([1, 9, 8, 4, 7, 10] if t == 0
                      else [N_T0_MUL + 8 * (t - 1) + 1, N_T0_MUL + 8 * (t - 1) + 3,
                            N_T0_MUL + 8 * (t - 1) + 4, N_T0_MUL + 8 * (t - 1) + 5,
                            N_T0_MUL + 8 * (t - 1) + 6, N_T0_MUL + 8 * (t - 1) + 8])
                groups = [
                    # (psum slot, taps, s_mul wait, s_mulp wait, start, stop)
                    (0, [(k, ident, None) for k in range(6, 9)],
                     mw[0], None, True, False),
                    (1, [(k, ident, None) for k in range(9)],
                     mw[1], None, True, True),
                    (0, [(k, ident, None) for k in range(3, 6)],
                     mw[2], None, False, False),
                    (0, [(k, wup, k) for k in range(3)],
                     mw[3], None, False, True),
                    (2, [(k, ident, None) for k in range(9)],
                     mw[4], 4 * t + 4, True, True),
                    (3, [(k, ident, None) for k in range(6)]
                        + [(k, wdn, 3 + k - 6) for k in range(6, 9)],
                     mw[5], None, True, True),
                ]
                for sg, taps, mw, mpw, st, sp in groups:
                    if mw is not None:
                        pe.wait_ge(s_mul, mw)
                    if mpw is not None:
                        pe.wait_ge(s_mulp, mpw)
                    if st:
                        # psum(sg, t%2) must be drained by its previous reader
                        if t == 0:
                            pe.wait_ge(s_ln, sg + 1)
                        elif t == 1:
                            pe.wait_ge(s_cev, sg + 1)
                        else:
                            pe.wait_ge(s_ev, 4 * (t - 2) + EVPOS[sg])
                    n = len(taps)
                    for j, (k, w_t, crow) in enumerate(taps):
                        kg_, kj_ = gk(k)
                        dj = OFFS[k][1]
                        if sg == 2:
                            mv = pr2.ap()[:, t % 2, kg_, kj_, 1 + dj:513 + dj]
                        elif crow is None:
                            mv = pr.ap()[:, kg_, kj_, sg, 1 + dj:513 + dj]
                        else:
                            mv = prc.ap()[:, crow, 1 + dj:513 + dj]
                        inst = pe.matmul(
                            pg(t % 2, sg)[:, 0:512], w_t.ap(), mv,
                            start=(st and j == 0), stop=(sp and j == n - 1),
                            skip_group_check=True,
                        )
                        if j == n - 1:
                            inst.then_inc(s_pe, 1)

        # ---------------- ACT: ln/exp normalization, evictions ---------------
        @block.scalar
        def _(sc):
            sc.wait_ge(s_hz, 6)   # c_eps ready
            for s in range(NS):
                sc.wait_ge(s_apex, s + 1)
                sc.activation(
                    asb.ap()[:, s, 1:513], pg(0, s)[:, 0:512], AF.Ln,
                    bias=c_eps.ap(),
                ).then_inc(s_ln, 1)
                sc.wait_ge(s_ln, s + 1)
                sc.activation(
                    rr.ap()[:, s, 1:513], asb.ap()[:, s, 1:513], AF.Exp,
                    scale=-1.0,
                ).then_inc(s_rexp, 1)
            for s in range(NS):
                sc.wait_ge(s_cpe, s + 1)
                sc.activation(
                    gt.ap()[:, 1, 1, s, 1:513], pg(1, s)[:, 0:512], AF.Identity,
                    bias=1.0, scale=-1.0,
                ).then_inc(s_cev, 1)
            # eviction order [1, 0, 2, 3] matches PE stop-group order
            pe_stop = {1: 2, 0: 4, 2: 5, 3: 6}
            # DVE cycle-(t-1) readers of h slot sg (last reading instr index)
            war_mul = {0: 8, 1: 2, 2: 7, 3: 7}
            war_mulp = {1: 1, 2: 2, 3: 4}
            for t in range(niter):
                hnext = hb[(t + 1) % 2]
                last = (t == niter - 1)
                for sg in EVORD:
                    sc.wait_ge(s_pe, 6 * t + pe_stop[sg])
                    if t == 0 and sg == EVORD[0]:
                        sc.wait_ge(s_hz, 8)   # h1 pad memsets done
                    if t > 0:
                        # h_next WAR: cycle t-1 readers of this buffer+slot
                        sc.wait_ge(s_mul, N_T0_MUL if t == 1
                                   else N_T0_MUL + 8 * (t - 2) + war_mul[sg])
                        if sg in war_mulp:
                            sc.wait_ge(s_mulp, 4 * (t - 1) + war_mulp[sg])
                    if last:
                        # final eviction straight to fp32 at true magnitude
                        # (undoes the 23 earlier rescales plus this one)
                        sc.activation(
                            asb.ap()[:, sg, 1:513],
                            pg(t % 2, sg)[:, 0:512], AF.Copy,
                            scale=RESCALE * float(1.0 / RESCALE) ** niter,
                        ).then_inc(s_fin, 1)
                    else:
                        sc.activation(
                            hnext.ap()[:, sg, 1:513],
                            pg(t % 2, sg)[:, 0:512], AF.Copy,
                            scale=RESCALE,
                        ).then_inc(s_ev, 1)

        # ---------------- SYNC: gtc shift DMAs + output ----------------------
        @block.sync
        def _(sy):
            # cross weights: one-time partition shifts of the normalized gt
            sy.wait_ge(s_gtd, 3)
            sy.dma_start(gtc.ap()[0:127, 0:3, 0:514],
                         gt.ap()[1:128, 0, :, 0, 0:514]).then_inc(s_gtcu, 16)
            sy.dma_start(gtc.ap()[127:128, 0:3, 0:514],
                         gt.ap()[0:1, 0, :, 0, 0:514]).then_inc(s_gtcu, 16)
            sy.wait_ge(s_gtd, len(DVE_SETUP_KS) * NS)
            sy.wait_ge(s_gtp, len(POOL_SETUP_KS) * NS)
            sy.dma_start(gtc.ap()[1:128, 3:6, 0:514],
                         gt.ap()[0:127, 2, :, 3, 0:514]).then_inc(s_gtcd, 16)
            sy.dma_start(gtc.ap()[0:1, 3:6, 0:514],
                         gt.ap()[127:128, 2, :, 3, 0:514]).then_inc(s_gtcd, 16)
            # per-slot output DMAs, each as soon as its final eviction lands
            for i, sg in enumerate(EVORD):
                sy.wait_ge(s_fin, i + 1)
                o_out = bass.AP(o_dram, sg * W, [[4 * W, 128], [1, W]])
                sy.dma_start(o_out, asb.ap()[:, sg, 1:513]).then_inc(s_out, 16)

    return nc


_NC_CACHE = {}


def kernel(guidance: np.ndarray, x: np.ndarray) -> np.ndarray:
    """guidance [8,8,512,512] f32, x [8,1,512,512] f32 -> [8,1,512,512] f32."""
    guidance = np.ascontiguousarray(np.asarray(guidance, dtype=np.float32))
    x = np.ascontiguousarray(np.asarray(x, dtype=np.float32))
    if "nc" not in _NC_CACHE:
        _NC_CACHE["nc"] = build_program()
    nc = _NC_CACHE["nc"]
    in_maps = [
        {"guidance": guidance[b], "x": x[b].reshape(1, H, W)}
        for b in range(N_CORES)
    ]
    res = run_bass_kernel_spmd(nc, in_maps, core_ids=list(range(N_CORES)))
    out = np.stack([res.results[b]["out"] for b in range(N_CORES)], axis=0)
    return out.astype(np.float32)


# revision 7
# speedup vs baseline: 1.0203x; 1.0065x over previous
"""CSPN (convolutional spatial propagation) Trainium2 kernel, v2.

Full inputs:  guidance [8, 8, 512, 512] f32, x [8, 1, 512, 512] f32.
Sharding: data-parallel over batch -- core b gets batch element b.

v2 redesign vs the halo-DMA baseline:
  * No halo rows at all.  h is [128, 4, PW]: partition p holds image rows
    4p..4p+3 (slots 0-3).  The 6 tap-rows whose input row lives on a
    neighbouring partition (slot0 di=-1, slot3 di=+1) are multiplied ON the
    partition that owns the h row, against one-time partition-shifted copies
    of the weights (gtc).  The PE re-aligns those products to the output
    partition with shifted-identity stationary matrices (free in both the
    cost model and -- via dual weight buffers -- nearly free on HW).
  * Product rows are split DVE 29 / Pool 7 per iteration (the real
    TRN2 ISA allows only plain tensor_tensor on Pool -- the faster
    TensorScalarPtr family is DVE-only).  Setup scale-muls all run
    on DVE in 2x mode (rr stored f16); PE p-state is kept warm with
    dummy-matmul keepalive blocks.
  * DVE products are merged into 5 wide instructions per iteration
    ([2-3, 3, 514] APs with double-broadcast h), cutting per-instruction
    overhead; Pool does one [3, 3, 514] instruction (slot 2).
  * Steady state is PE-bound at 36 matmul-columns x 512 per iteration
    (~7.7 us); DVE ~7.5 us, Pool ~6.5 us, ACT ~2.5 us.

Numerics are identical to the baseline: fp16 weights/products, fp32 PSUM
accumulation, center gate gt4 = 1 - sum(fp16-rounded gt_k), h rescaled by
0.5/iter to stay in fp16 range, un-scaled by 2^24 in the final fp32 pass.
"""

import sys

sys.path.insert(0, "/opt/trn_rl_repo")

import numpy as np

import concourse.bass as bass
from concourse import mybir
from concourse.bass_utils import run_bass_kernel_spmd
from concourse.alu_op_type import AluOpType

F16 = mybir.dt.float16
F32 = mybir.dt.float32
U16 = mybir.dt.uint16
AF = mybir.ActivationFunctionType

N_CORES = 8
H, W = 512, 512
NS = 4            # row-slots per partition
PW = 520          # f16 elements per row slot (514 used + pad)
NITER = 24
RESCALE = 0.5
# tap k = (di+1)*3 + (dj+1); k=4 is the center gate
OFFS = [(k // 3 - 1, k % 3 - 1) for k in range(9)]
# guidance channel for tap k (center has none)
CH_FOR_K = [0, 1, 2, 3, None, 4, 5, 6, 7]
# setup scale-muls: 6 on DVE (rr f16 -> 2x mode), 2 on the otherwise-idle
# Pool via plain tensor_tensor (the only mult op the real ISA allows there)
DVE_SETUP_KS = (0, 1, 2, 3, 5, 8)
POOL_SETUP_KS = (6, 7)
# eviction order within a cycle (follows the PE stop-group order) and the
# 1-based position of each slot's eviction in that order
EVORD = [1, 0, 2, 3]
EVPOS = {1: 1, 0: 2, 2: 3, 3: 4}
# DVE product-instruction count at t=0 (9: split center groups)
N_T0_MUL = 10
N_KEEPALIVE = 60
# column split of the slot-2 k7/k8 rows between Pool [0:CSPL] and DVE
CSPL = 120


def build_program(niter=NITER):
    nc = bass.Bass("TRN2", target_bir_lowering=False, debug=False)

    g_dram = nc.dram_tensor("guidance", [8, H, W], F32, kind="ExternalInput")
    x_dram = nc.dram_tensor("x", [1, H, W], F32, kind="ExternalInput")
    o_dram = nc.dram_tensor("out", [1, H, W], F32, kind="ExternalOutput")

    h0 = nc.alloc_sbuf_tensor("h0", [128, NS, PW], F16)
    h1 = nc.alloc_sbuf_tensor("h1", [128, NS, PW], F16)
    # weights / products indexed [kg, kj, slot, col] with tap k = 3*kg + kj
    gt = nc.alloc_sbuf_tensor("gt", [128, 3, 3, NS, PW], F16)
    pr = nc.alloc_sbuf_tensor("pr", [128, 3, 3, NS, PW], F16)
    # slot-2 products, double-buffered so Pool(t+1) never waits on PE2(t)
    pr2 = nc.alloc_sbuf_tensor("pr2", [128, 2, 3, 3, PW], F16)
    # cross rows: 0-2 = slot0 taps k=0,1,2 shifted up (gtc[p] = gt[p+1]);
    #             3-5 = slot3 taps k=6,7,8 shifted down (gtc[p] = gt[p-1])
    gtc = nc.alloc_sbuf_tensor("gtc", [128, 6, PW], F16)
    prc = nc.alloc_sbuf_tensor("prc", [128, 6, PW], F16)
    graw = nc.alloc_sbuf_tensor("graw", [128, 8, NS, PW], F16)
    gabs = nc.alloc_sbuf_tensor("gabs", [128, 2, 8, PW], F16)
    asb = nc.alloc_sbuf_tensor("asb", [128, NS, PW], F32)   # ln(A); out stage
    rr = nc.alloc_sbuf_tensor("rr", [128, NS, PW], F16)     # r = 1/A
    ident = nc.alloc_sbuf_tensor("ident", [128, 128], F16)
    wup = nc.alloc_sbuf_tensor("wup", [128, 128], F16)      # psum[p] += x[p-1]
    wdn = nc.alloc_sbuf_tensor("wdn", [128, 128], F16)      # psum[p] += x[p+1]
    c_eps = nc.alloc_sbuf_tensor("c_eps", [128, 1], F32)

    psum = [nc.alloc_psum_tensor(f"pg{g}s{s}", [128, W], F32)
            for g in range(2) for s in range(NS)]

    def pg(g, s):
        return psum[g * NS + s].ap()

    hb = [h0, h1]

    s_hz = nc.alloc_semaphore("s_hz")      # DVE memsets done
    s_id = nc.alloc_semaphore("s_id")      # stationaries built (3)
    s_x = nc.alloc_semaphore("s_x")        # x DMA (+16)
    s_gs = [nc.alloc_semaphore(f"s_g{i}") for i in range(NS)]
    s_abs = nc.alloc_semaphore("s_abs")    # DVE abs per slot
    s_apex = nc.alloc_semaphore("s_apex")  # PE A-sum per slot
    s_ln = nc.alloc_semaphore("s_ln")      # ACT ln per slot
    s_rexp = nc.alloc_semaphore("s_rexp")  # ACT exp per slot
    s_gtd = nc.alloc_semaphore("s_gtd")    # DVE setup muls (5/slot)
    s_gtp = nc.alloc_semaphore("s_gtp")    # Pool setup muls (3/slot)
    s_cpe = nc.alloc_semaphore("s_cpe")    # PE center sum per slot
    s_cev = nc.alloc_semaphore("s_cev")    # ACT center evict per slot
    s_gtcu = nc.alloc_semaphore("s_gtcu")  # gtc up-shift DMA (+16)
    s_gtcd = nc.alloc_semaphore("s_gtcd")  # gtc down-shift DMA (+16)
    s_mul = nc.alloc_semaphore("s_mul")    # DVE iter muls (+5/iter)
    s_mulp = nc.alloc_semaphore("s_mulp")  # Pool iter muls (+1/iter)
    s_pe = nc.alloc_semaphore("s_pe")      # PE tap-sum per slot (+4/iter)
    s_ev = nc.alloc_semaphore("s_ev")      # ACT evict per slot (+4/iter)
    s_fin = nc.alloc_semaphore("s_fin")    # final ACT rescale (4)
    s_out = nc.alloc_semaphore("s_out")    # output DMA

    N_MEMSET = 6

    def gk(k):
        return k // 3, k % 3

    def emit_dve_gtmuls(v, s):
        v.wait_ge(s_hz, 14)   # gt pad memsets retired (same-engine WAW)
        v.wait_ge(s_rexp, s + 1)
        for k in DVE_SETUP_KS:
            kg_, kj_ = gk(k)
            dj = OFFS[k][1]
            v.tensor_tensor(
                out=gt.ap()[:, kg_, kj_, s, 1 + dj:513 + dj],
                in0=graw.ap()[:, CH_FOR_K[k], s, 1:513],
                in1=rr.ap()[:, s, 1:513],
                op=AluOpType.mult,
            ).then_inc(s_gtd, 1)

    def h_bcast(h, lo, hi, width):
        """h slots [lo:hi] broadcast over the 3-tap (kj) dim."""
        n = hi - lo
        return (h.ap()[:, lo:hi, 0:514]
                .unsqueeze(2).broadcast_to([width, n, 3, 514]))

    with nc.Block() as block:

        # ---------------- GPSIMD/Pool: DMAs, stationaries, products ----------
        @block.gpsimd
        def _(gp):
            # guidance slot 0 first -- needs no memsets, so it starts at t=0
            g_in0 = bass.AP(g_dram, 0, [[4 * W, 128], [H * W, 8], [1, W]])
            gp.dma_start(graw.ap()[:, :, 0, 1:513], g_in0).then_inc(s_gs[0], 16)
            gp.wait_ge(s_hz, 3)
            for w_t, base in ((ident, 0), (wup, 1), (wdn, -1)):
                gp.affine_select(
                    out=w_t.ap(), in_=w_t.ap(),
                    compare_op=AluOpType.not_equal, fill=1.0, base=base,
                    pattern=[[-1, 128]], channel_multiplier=1,
                ).then_inc(s_id, 1)
            for s in range(1, NS):
                g_in = bass.AP(g_dram, s * W,
                               [[4 * W, 128], [H * W, 8], [1, W]])
                gp.dma_start(graw.ap()[:, :, s, 1:513], g_in).then_inc(s_gs[s], 16)
            # x last: it is not needed until the first products (~cev0 time)
            gp.wait_ge(s_hz, 5)
            x_in = bass.AP(x_dram, 0, [[4 * W, 128], [W, NS], [1, W]])
            gp.dma_start(h0.ap()[:, :, 1:513], x_in).then_inc(s_x, 16)
            # iteration products: slot 2, one stt per tap-triple (kg) so each
            # starts as soon as its h slot (1+kg) is evicted
            def pool_kg(t, hcur, kg_):
                gp.tensor_tensor(
                    out=pr2.ap()[:, t % 2, kg_, :, 0:514],
                    in0=gt.ap()[:, kg_, :, 2, 0:514],
                    in1=(hcur.ap()[:, 1 + kg_:2 + kg_, 0:514]
                         .broadcast_to([128, 3, 514])),
                    op=AluOpType.mult,
                ).then_inc(s_mulp, 1)

            def pool_k6(t, hcur):
                gp.tensor_tensor(
                    out=pr2.ap()[:, t % 2, 2, 0, 0:514],
                    in0=gt.ap()[:, 2, 0, 2, 0:514],
                    in1=hcur.ap()[:, 3, 0:514],
                    op=AluOpType.mult,
                ).then_inc(s_mulp, 1)

            def pool_k78s(t, hcur):
                gp.tensor_tensor(
                    out=pr2.ap()[:, t % 2, 2, 1:3, 0:CSPL],
                    in0=gt.ap()[:, 2, 1:3, 2, 0:CSPL],
                    in1=(hcur.ap()[:, 3:4, 0:CSPL]
                         .broadcast_to([128, 2, CSPL])),
                    op=AluOpType.mult,
                ).then_inc(s_mulp, 1)

            # setup scale-muls for taps k=6,7 of each slot
            gp.wait_ge(s_hz, 14)   # gt pad memsets (WAW on shared columns)
            for s in range(NS):
                gp.wait_ge(s_rexp, s + 1)
                for k in POOL_SETUP_KS:
                    kg_, kj_ = gk(k)
                    dj = OFFS[k][1]
                    gp.tensor_tensor(
                        out=gt.ap()[:, kg_, kj_, s, 1 + dj:513 + dj],
                        in0=graw.ap()[:, CH_FOR_K[k], s, 1:513],
                        in1=rr.ap()[:, s, 1:513],
                        op=AluOpType.mult,
                    ).then_inc(s_gtp, 1)
            # t=0: kg1 holds the center tap (needs cev2); run it last
            gp.wait_ge(s_gtd, 18)
            gp.wait_ge(s_gtp, 8)   # own setup-mul writes retired
            gp.wait_ge(s_x, 16)
            pool_kg(0, h0, 0)
            pool_k6(0, h0)
            pool_k78s(0, h0)
            gp.wait_ge(s_cev, 3)
            pool_kg(0, h0, 1)
            for t in range(1, niter):
                hcur = hb[t % 2]
                gp.wait_ge(s_ev, 4 * (t - 1) + EVPOS[1])
                if t >= 2:
                    gp.wait_ge(s_pe, 6 * (t - 2) + 5)
                pool_kg(t, hcur, 0)
                gp.wait_ge(s_ev, 4 * (t - 1) + EVPOS[2])
                pool_kg(t, hcur, 1)
                gp.wait_ge(s_ev, 4 * (t - 1) + EVPOS[3])
                pool_k6(t, hcur)
                pool_k78s(t, hcur)
            gp.wait_ge(s_out, 64)

        # ---------------- DVE: memsets, abs, setup muls, products ------------
        @block.vector
        def _(v):
            v.memset(ident.ap(), 0.0).then_inc(s_hz, 1)
            v.memset(wup.ap(), 0.0).then_inc(s_hz, 1)
            v.memset(wdn.ap(), 0.0).then_inc(s_hz, 1)
            # h pad columns only; bodies are written by the x DMA (h0) and
            # by evictions (h1) before any read
            v.memset(h0.ap()[:, :, 0:1], 0.0).then_inc(s_hz, 1)
            v.memset(h0.ap()[:, :, 513:520], 0.0).then_inc(s_hz, 1)
            v.memset(c_eps.ap(), 1e-8).then_inc(s_hz, 1)
            v.memset(h1.ap()[:, :, 0:1], 0.0).then_inc(s_hz, 1)
            v.memset(h1.ap()[:, :, 513:520], 0.0).then_inc(s_hz, 1)
            # abs per slot as guidance DMAs land; pad memsets fill the gaps
            # (their consumers are transitively gated through s_gtd/s_mul)
            for s in range(NS):
                v.wait_ge(s_gs[s], 16)
                if s >= 2:
                    v.wait_ge(s_apex, s - 1)  # gabs[s%2] free again
                v.tensor_scalar(
                    out=gabs.ap()[:, s % 2, :, 1:513].bitcast(U16),
                    in0=graw.ap()[:, :, s, 1:513].bitcast(U16),
                    scalar1=0x7FFF, scalar2=None, op0=AluOpType.bitwise_and,
                ).then_inc(s_abs, 1)
                if s == 0:
                    for kg_ in range(3):
                        v.memset(gt.ap()[:, kg_, :, :, 0:2], 0.0).then_inc(s_hz, 1)
                        v.memset(gt.ap()[:, kg_, :, :, 512:520], 0.0).then_inc(s_hz, 1)
                else:
                    # slot s-1 scale-muls, interleaved so they start as soon
                    # as exp(s-1) lands instead of after all four abs passes
                    emit_dve_gtmuls(v, s - 1)
            emit_dve_gtmuls(v, 3)
            # iteration products: 7 instructions per iteration, ordered by
            # gate availability then consumer deadline.  PE window order:
            # 0a'(k678), slot1, 0a''(k345), 0b(cross), slot2, slot3.
            # t=0 uses a different order and precise setup gates (only the
            # tap-groups containing the center k=4 need the center-evict).
            def p_s0kg2(t, hcur):
                v.tensor_tensor(
                    out=pr.ap()[:, 2, :, 0, 0:514],
                    in0=gt.ap()[:, 2, :, 0, 0:514],
                    in1=(hcur.ap()[:, 1:2, 0:514]
                         .broadcast_to([128, 3, 514])),
                    op=AluOpType.mult,
                ).then_inc(s_mul, 1)

            def p_s1kg01(t, hcur):
                v.tensor_tensor(
                    out=pr.ap()[:, 0:2, :, 1, 0:514],
                    in0=gt.ap()[:, 0:2, :, 1, 0:514],
                    in1=h_bcast(hcur, 0, 2, 128), op=AluOpType.mult,
                ).then_inc(s_mul, 1)

            def p_s1kg2(t, hcur):
                v.tensor_tensor(
                    out=pr.ap()[:, 2, :, 1, 0:514],
                    in0=gt.ap()[:, 2, :, 1, 0:514],
                    in1=(hcur.ap()[:, 2:3, 0:514]
                         .broadcast_to([128, 3, 514])),
                    op=AluOpType.mult,
                ).then_inc(s_mul, 1)

            def p_s0kg1(t, hcur):
                v.tensor_tensor(
                    out=pr.ap()[:, 1, :, 0, 0:514],
                    in0=gt.ap()[:, 1, :, 0, 0:514],
                    in1=(hcur.ap()[:, 0:1, 0:514]
                         .broadcast_to([128, 3, 514])),
                    op=AluOpType.mult,
                ).then_inc(s_mul, 1)

            def p_up3(t, hcur):
                v.tensor_tensor(
                    out=prc.ap()[:, 0:3, 0:514],
                    in0=gtc.ap()[:, 0:3, 0:514],
                    in1=(hcur.ap()[:, 3:4, 0:514]
                         .broadcast_to([128, 3, 514])),
                    op=AluOpType.mult,
                ).then_inc(s_mul, 1)

            def p_s3kg01(t, hcur):
                v.tensor_tensor(
                    out=pr.ap()[:, 0:2, :, 3, 0:514],
                    in0=gt.ap()[:, 0:2, :, 3, 0:514],
                    in1=h_bcast(hcur, 2, 4, 128), op=AluOpType.mult,
                ).then_inc(s_mul, 1)

            def p_s2k78(t, hcur):
                # cols [CSPL:514]; Pool covers [0:CSPL] of the same rows
                v.tensor_tensor(
                    out=pr2.ap()[:, t % 2, 2, 1:3, CSPL:514],
                    in0=gt.ap()[:, 2, 1:3, 2, CSPL:514],
                    in1=(hcur.ap()[:, 3:4, CSPL:514]
                         .broadcast_to([128, 2, 514 - CSPL])),
                    op=AluOpType.mult,
                ).then_inc(s_mul, 1)

            def p_dn3(t, hcur):
                v.tensor_tensor(
                    out=prc.ap()[:, 3:6, 0:514],
                    in0=gtc.ap()[:, 3:6, 0:514],
                    in1=(hcur.ap()[:, 0:1, 0:514]
                         .broadcast_to([128, 3, 514])),
                    op=AluOpType.mult,
                ).then_inc(s_mul, 1)

            # t=0: split the center-bearing tap-groups so only three tail
            # instructions wait on center-evicts; everything else is gated by
            # the scale-muls / gtc shifts alone
            def p_kg(sg, kg_, hslot):
                v.tensor_tensor(
                    out=pr.ap()[:, kg_, :, sg, 0:514],
                    in0=gt.ap()[:, kg_, :, sg, 0:514],
                    in1=(h0.ap()[:, hslot:hslot + 1, 0:514]
                         .broadcast_to([128, 3, 514])),
                    op=AluOpType.mult,
                ).then_inc(s_mul, 1)

            v.wait_ge(s_gtd, 6)
            v.wait_ge(s_gtp, 2)
            v.wait_ge(s_x, 16)
            p_s0kg2(0, h0)            # 1
            v.wait_ge(s_gtd, 12)
            v.wait_ge(s_gtp, 4)
            p_kg(1, 0, 0)             # 2: slot1 k0-2
            p_s1kg2(0, h0)            # 3
            v.wait_ge(s_gtcu, 32)
            p_up3(0, h0)              # 4
            v.wait_ge(s_gtcd, 32)
            p_dn3(0, h0)              # 5
            v.wait_ge(s_gtd, 24)
            p_kg(3, 0, 2)             # 6: slot3 k0-2
            v.wait_ge(s_gtp, 6)
            p_s2k78(0, h0)            # 7: slot2 k7,k8
            v.wait_ge(s_cev, 1)
            p_kg(0, 1, 0)             # 8: slot0 k3-5 (center)
            v.wait_ge(s_cev, 2)
            p_kg(1, 1, 1)             # 9: slot1 k3-5 (center)
            v.wait_ge(s_cev, 4)
            p_kg(3, 1, 3)             # 10: slot3 k3-5 (center)
            # t>=1: steady-state order and gates
            for t in range(1, niter):
                hcur = hb[t % 2]
                v.wait_ge(s_ev, 4 * (t - 1) + 1)
                v.wait_ge(s_pe, 6 * (t - 1) + 1)
                p_s0kg2(t, hcur)
                v.wait_ge(s_ev, 4 * (t - 1) + 2)
                v.wait_ge(s_pe, 6 * (t - 1) + 2)
                p_s1kg01(t, hcur)
                v.wait_ge(s_ev, 4 * (t - 1) + 3)
                p_s1kg2(t, hcur)
                v.wait_ge(s_pe, 6 * (t - 1) + 3)
                p_s0kg1(t, hcur)
                v.wait_ge(s_ev, 4 * (t - 1) + 4)
                v.wait_ge(s_pe, 6 * (t - 1) + 4)
                p_up3(t, hcur)
                if t >= 2:
                    v.wait_ge(s_pe, 6 * (t - 2) + 5)   # pr2 parity WAR
                p_s2k78(t, hcur)
                v.wait_ge(s_pe, 6 * (t - 1) + 6)
                p_s3kg01(t, hcur)
                p_dn3(t, hcur)

        # ---------------- PE: setup sums + iteration tap-sums ----------------
        @block.tensor
        def _(pe):
            pe.wait_ge(s_id, 3)
            # warm-up: ramp the PE p-state past the 3us threshold on dummy
            # 128-col matmuls so the first A-sum runs at full clock
            for _ in range(30):
                pe.matmul(pg(1, 0)[:, 0:128], ident.ap(), ident.ap(),
                          start=True, stop=True, skip_group_check=True)
            # A = sum_ch |g_ch| into psum group 0
            for s in range(NS):
                pe.wait_ge(s_abs, s + 1)
                for ch in range(8):
                    inst = pe.matmul(
                        pg(0, s)[:, 0:512], ident.ap(),
                        gabs.ap()[:, s % 2, ch, 1:513],
                        start=(ch == 0), stop=(ch == 7),
                    )
                    if ch == 7:
                        inst.then_inc(s_apex, 1)
            # center gate: sum of fp16-rounded weights into psum group 1
            for s in range(NS):
                pe.wait_ge(s_gtd, len(DVE_SETUP_KS) * (s + 1))
                pe.wait_ge(s_gtp, len(POOL_SETUP_KS) * (s + 1))
                done = 0
                for k in range(9):
                    if k == 4:
                        continue
                    kg_, kj_ = gk(k)
                    dj = OFFS[k][1]
                    inst = pe.matmul(
                        pg(1, s)[:, 0:512], ident.ap(),
                        gt.ap()[:, kg_, kj_, s, 1 + dj:513 + dj],
                        start=(done == 0), stop=(done == 7),
                    )
                    done += 1
                    if done == 8:
                        inst.then_inc(s_cpe, 1)
            # keepalive: bridge the idle gap between setup and the first
            # window so the p-state ramp survives into T0's matmuls.
            # pg(0,0)'s only setup reader (ln0) is gated done via s_ln.
            pe.wait_ge(s_ln, 1)
            for _ in range(N_KEEPALIVE):
                pe.matmul(pg(0, 0)[:, 0:128], ident.ap(), ident.ap(),
                          start=True, stop=True, skip_group_check=True)
            # iterations: window order 0a'(k678, start psum0), slot1,
            # 0a''(k345), 0b(cross, stop psum0), slot2, slot3;
            # s_pe +1 after each of the 6 groups
            for t in range(niter):
                # t=0 DVE emission order differs; map each group to the
                # s_mul count of its last-needed product
                mw = ([1, 9, 8, 4, 7, 10] if t == 0
                      else [N_T0_MUL + 8 * (t - 1) + 1, N_T0_MUL + 8 * (t - 1) + 3,
                            N_T0_MUL + 8 * (t - 1) + 4, N_T0_MUL + 8 * (t - 1) + 5,
                            N_T0_MUL + 8 * (t - 1) + 6, N_T0_MUL + 8 * (t - 1) + 8])
                groups = [
                    # (psum slot, taps, s_mul wait, s_mulp wait, start, stop)
                    (0, [(k, ident, None) for k in range(6, 9)],
                     mw[0], None, True, False),
                    (1, [(k, ident, None) for k in range(9)],
                     mw[1], None, True, True),
                    (0, [(k, ident, None) for k in range(3, 6)],
                     mw[2], None, False, False),
                    (0, [(k, wup, k) for k in range(3)],
                     mw[3], None, False, True),
                    (2, [(k, ident, None) for k in range(9)],
                     mw[4], 4 * t + 4, True, True),
                    (3, [(k, ident, None) for k in range(6)]
                        + [(k, wdn, 3 + k - 6) for k in range(6, 9)],
                     mw[5], None, True, True),
                ]
                for sg, taps, mw, mpw, st, sp in groups:
                    if mw is not None:
                        pe.wait_ge(s_mul, mw)
                    if mpw is not None:
                        pe.wait_ge(s_mulp, mpw)
                    if st:
                        # psum(sg, t%2) must be drained by its previous reader
                        if t == 0:
                            pe.wait_ge(s_ln, sg + 1)
                        elif t == 1:
                            pe.wait_ge(s_cev, sg + 1)
                        else:
                            pe.wait_ge(s_ev, 4 * (t - 2) + EVPOS[sg])
                    n = len(taps)
                    for j, (k, w_t, crow) in enumerate(taps):
                        kg_, kj_ = gk(k)
                        dj = OFFS[k][1]
                        if sg == 2:
                            mv = pr2.ap()[:, t % 2, kg_, kj_, 1 + dj:513 + dj]
                        elif crow is None:
                            mv = pr.ap()[:, kg_, kj_, sg, 1 + dj:513 + dj]
                        else:
                            mv = prc.ap()[:, crow, 1 + dj:513 + dj]
                        inst = pe.matmul(
                            pg(t % 2, sg)[:, 0:512], w_t.ap(), mv,
                            start=(st and j == 0), stop=(sp and j == n - 1),
                            skip_group_check=True,
                        )
                        if j == n - 1:
                            inst.then_inc(s_pe, 1)

        # ---------------- ACT: ln/exp normalization, evictions ---------------
        @block.scalar
        def _(sc):
            sc.wait_ge(s_hz, 6)   # c_eps ready
            for s in range(NS):
                sc.wait_ge(s_apex, s + 1)
                sc.activation(
                    asb.ap()[:, s, 1:513], pg(0, s)[:, 0:512], AF.Ln,
                    bias=c_eps.ap(),
                ).then_inc(s_ln, 1)
                sc.wait_ge(s_ln, s + 1)
                sc.activation(
                    rr.ap()[:, s, 1:513], asb.ap()[:, s, 1:513], AF.Exp,
                    scale=-1.0,
                ).then_inc(s_rexp, 1)
            for s in range(NS):
                sc.wait_ge(s_cpe, s + 1)
                sc.activation(
                    gt.ap()[:, 1, 1, s, 1:513], pg(1, s)[:, 0:512], AF.Identity,
                    bias=1.0, scale=-1.0,
                ).then_inc(s_cev, 1)
            # eviction order [1, 0, 2, 3] matches PE stop-group order
            pe_stop = {1: 2, 0: 4, 2: 5, 3: 6}
            # DVE cycle-(t-1) readers of h slot sg (last reading instr index)
            war_mul = {0: 8, 1: 2, 2: 7, 3: 7}
            war_mulp = {1: 1, 2: 2, 3: 4}
            for t in range(niter):
                hnext = hb[(t + 1) % 2]
                last = (t == niter - 1)
                for sg in EVORD:
                    sc.wait_ge(s_pe, 6 * t + pe_stop[sg])
                    if t == 0 and sg == EVORD[0]:
                        sc.wait_ge(s_hz, 8)   # h1 pad memsets done
                    if t > 0:
                        # h_next WAR: cycle t-1 readers of this buffer+slot
                        sc.wait_ge(s_mul, N_T0_MUL if t == 1
                                   else N_T0_MUL + 8 * (t - 2) + war_mul[sg])
                        if sg in war_mulp:
                            sc.wait_ge(s_mulp, 4 * (t - 1) + war_mulp[sg])
                    if last:
                        # final eviction straight to fp32 at true magnitude
                        # (undoes the 23 earlier rescales plus this one)
                        sc.activation(
                            asb.ap()[:, sg, 1:513],
                            pg(t % 2, sg)[:, 0:512], AF.Copy,
                            scale=RESCALE * float(1.0 / RESCALE) ** niter,
                        ).then_inc(s_fin, 1)
                    else:
                        sc.activation(
                            hnext.ap()[:, sg, 1:513],
                            pg(t % 2, sg)[:, 0:512], AF.Copy,
                            scale=RESCALE,
                        ).then_inc(s_ev, 1)

        # ---------------- SYNC: gtc shift DMAs + output ----------------------
        @block.sync
        def _(sy):
            # cross weights: one-time partition shifts of the normalized gt
            sy.wait_ge(s_gtd, 3)
            sy.dma_start(gtc.ap()[0:127, 0:3, 0:514],
                         gt.ap()[1:128, 0, :, 0, 0:514]).then_inc(s_gtcu, 16)
            sy.dma_start(gtc.ap()[127:128, 0:3, 0:514],
                         gt.ap()[0:1, 0, :, 0, 0:514]).then_inc(s_gtcu, 16)
            sy.wait_ge(s_gtd, len(DVE_SETUP_KS) * NS)
            sy.wait_ge(s_gtp, len(POOL_SETUP_KS) * NS)
            sy.dma_start(gtc.ap()[1:128, 3:6, 0:514],
                         gt.ap()[0:127, 2, :, 3, 0:514]).then_inc(s_gtcd, 16)
            sy.dma_start(gtc.ap()[0:1, 3:6, 0:514],
                         gt.ap()[127:128, 2, :, 3, 0:514]).then_inc(s_gtcd, 16)
            # per-slot output DMAs, each as soon as its final eviction lands
            for i, sg in enumerate(EVORD):
                sy.wait_ge(s_fin, i + 1)
                o_out = bass.AP(o_dram, sg * W, [[4 * W, 128], [1, W]])
                sy.dma_start(o_out, asb.ap()[:, sg, 1:513]).then_inc(s_out, 16)

    return nc


_NC_CACHE = {}


def kernel(guidance: np.ndarray, x: np.ndarray) -> np.ndarray:
    """guidance [8,8,512,512] f32, x [8,1,512,512] f32 -> [8,1,512,512] f32."""
    guidance = np.ascontiguousarray(np.asarray(guidance, dtype=np.float32))
    x = np.ascontiguousarray(np.asarray(x, dtype=np.float32))
    if "nc" not in _NC_CACHE:
        _NC_CACHE["nc"] = build_program()
    nc = _NC_CACHE["nc"]
    in_maps = [
        {"guidance": guidance[b], "x": x[b].reshape(1, H, W)}
        for b in range(N_CORES)
    ]
    res = run_bass_kernel_spmd(nc, in_maps, core_ids=list(range(N_CORES)))
    out = np.stack([res.results[b]["out"] for b in range(N_CORES)], axis=0)
    return out.astype(np.float32)


# revision 8
# speedup vs baseline: 1.0209x; 1.0006x over previous
"""CSPN (convolutional spatial propagation) Trainium2 kernel, v2.

Full inputs:  guidance [8, 8, 512, 512] f32, x [8, 1, 512, 512] f32.
Sharding: data-parallel over batch -- core b gets batch element b.

v2 redesign vs the halo-DMA baseline:
  * No halo rows at all.  h is [128, 4, PW]: partition p holds image rows
    4p..4p+3 (slots 0-3).  The 6 tap-rows whose input row lives on a
    neighbouring partition (slot0 di=-1, slot3 di=+1) are multiplied ON the
    partition that owns the h row, against one-time partition-shifted copies
    of the weights (gtc).  The PE re-aligns those products to the output
    partition with shifted-identity stationary matrices (free in both the
    cost model and -- via dual weight buffers -- nearly free on HW).
  * Product rows are split DVE 29 / Pool 7 per iteration (the real
    TRN2 ISA allows only plain tensor_tensor on Pool -- the faster
    TensorScalarPtr family is DVE-only).  Setup scale-muls all run
    on DVE in 2x mode (rr stored f16); PE p-state is kept warm with
    dummy-matmul keepalive blocks.
  * DVE products are merged into 5 wide instructions per iteration
    ([2-3, 3, 514] APs with double-broadcast h), cutting per-instruction
    overhead; Pool does one [3, 3, 514] instruction (slot 2).
  * Steady state is PE-bound at 36 matmul-columns x 512 per iteration
    (~7.7 us); DVE ~7.5 us, Pool ~6.5 us, ACT ~2.5 us.

Numerics are identical to the baseline: fp16 weights/products, fp32 PSUM
accumulation, center gate gt4 = 1 - sum(fp16-rounded gt_k), h rescaled by
0.5/iter to stay in fp16 range, un-scaled by 2^24 in the final fp32 pass.
"""

import sys

sys.path.insert(0, "/opt/trn_rl_repo")

import numpy as np

import concourse.bass as bass
from concourse import mybir
from concourse.bass_utils import run_bass_kernel_spmd
from concourse.alu_op_type import AluOpType

F16 = mybir.dt.float16
F32 = mybir.dt.float32
U16 = mybir.dt.uint16
AF = mybir.ActivationFunctionType

N_CORES = 8
H, W = 512, 512
NS = 4            # row-slots per partition
PW = 520          # f16 elements per row slot (514 used + pad)
NITER = 24
RESCALE = 0.5
# tap k = (di+1)*3 + (dj+1); k=4 is the center gate
OFFS = [(k // 3 - 1, k % 3 - 1) for k in range(9)]
# guidance channel for tap k (center has none)
CH_FOR_K = [0, 1, 2, 3, None, 4, 5, 6, 7]
# setup scale-muls: 6 on DVE (rr f16 -> 2x mode), 2 on the otherwise-idle
# Pool via plain tensor_tensor (the only mult op the real ISA allows there)
DVE_SETUP_KS = (0, 1, 2, 3, 5, 8)
POOL_SETUP_KS = (6, 7)
# eviction order within a cycle (follows the PE stop-group order) and the
# 1-based position of each slot's eviction in that order
EVORD = [1, 0, 2, 3]
EVPOS = {1: 1, 0: 2, 2: 3, 3: 4}
# DVE product-instruction count at t=0 (9: split center groups)
N_T0_MUL = 10
N_KEEPALIVE = 60
# column split of the slot-2 k7/k8 rows between Pool [0:CSPL] and DVE
CSPL = 126


def build_program(niter=NITER):
    nc = bass.Bass("TRN2", target_bir_lowering=False, debug=False)

    g_dram = nc.dram_tensor("guidance", [8, H, W], F32, kind="ExternalInput")
    x_dram = nc.dram_tensor("x", [1, H, W], F32, kind="ExternalInput")
    o_dram = nc.dram_tensor("out", [1, H, W], F32, kind="ExternalOutput")

    h0 = nc.alloc_sbuf_tensor("h0", [128, NS, PW], F16)
    h1 = nc.alloc_sbuf_tensor("h1", [128, NS, PW], F16)
    # weights / products indexed [kg, kj, slot, col] with tap k = 3*kg + kj
    gt = nc.alloc_sbuf_tensor("gt", [128, 3, 3, NS, PW], F16)
    pr = nc.alloc_sbuf_tensor("pr", [128, 3, 3, NS, PW], F16)
    # slot-2 products, double-buffered so Pool(t+1) never waits on PE2(t)
    pr2 = nc.alloc_sbuf_tensor("pr2", [128, 2, 3, 3, PW], F16)
    # cross rows: 0-2 = slot0 taps k=0,1,2 shifted up (gtc[p] = gt[p+1]);
    #             3-5 = slot3 taps k=6,7,8 shifted down (gtc[p] = gt[p-1])
    gtc = nc.alloc_sbuf_tensor("gtc", [128, 6, PW], F16)
    prc = nc.alloc_sbuf_tensor("prc", [128, 6, PW], F16)
    graw = nc.alloc_sbuf_tensor("graw", [128, 8, NS, PW], F16)
    gabs = nc.alloc_sbuf_tensor("gabs", [128, 2, 8, PW], F16)
    asb = nc.alloc_sbuf_tensor("asb", [128, NS, PW], F32)   # ln(A); out stage
    rr = nc.alloc_sbuf_tensor("rr", [128, NS, PW], F16)     # r = 1/A
    ident = nc.alloc_sbuf_tensor("ident", [128, 128], F16)
    wup = nc.alloc_sbuf_tensor("wup", [128, 128], F16)      # psum[p] += x[p-1]
    wdn = nc.alloc_sbuf_tensor("wdn", [128, 128], F16)      # psum[p] += x[p+1]
    c_eps = nc.alloc_sbuf_tensor("c_eps", [128, 1], F32)

    psum = [nc.alloc_psum_tensor(f"pg{g}s{s}", [128, W], F32)
            for g in range(2) for s in range(NS)]

    def pg(g, s):
        return psum[g * NS + s].ap()

    hb = [h0, h1]

    s_hz = nc.alloc_semaphore("s_hz")      # DVE memsets done
    s_id = nc.alloc_semaphore("s_id")      # stationaries built (3)
    s_x = nc.alloc_semaphore("s_x")        # x DMA (+16)
    s_gs = [nc.alloc_semaphore(f"s_g{i}") for i in range(NS)]
    s_abs = nc.alloc_semaphore("s_abs")    # DVE abs per slot
    s_apex = nc.alloc_semaphore("s_apex")  # PE A-sum per slot
    s_ln = nc.alloc_semaphore("s_ln")      # ACT ln per slot
    s_rexp = nc.alloc_semaphore("s_rexp")  # ACT exp per slot
    s_gtd = nc.alloc_semaphore("s_gtd")    # DVE setup muls (5/slot)
    s_gtp = nc.alloc_semaphore("s_gtp")    # Pool setup muls (3/slot)
    s_cpe = nc.alloc_semaphore("s_cpe")    # PE center sum per slot
    s_cev = nc.alloc_semaphore("s_cev")    # ACT center evict per slot
    s_gtcu = nc.alloc_semaphore("s_gtcu")  # gtc up-shift DMA (+16)
    s_gtcd = nc.alloc_semaphore("s_gtcd")  # gtc down-shift DMA (+16)
    s_mul = nc.alloc_semaphore("s_mul")    # DVE iter muls (+5/iter)
    s_mulp = nc.alloc_semaphore("s_mulp")  # Pool iter muls (+1/iter)
    s_pe = nc.alloc_semaphore("s_pe")      # PE tap-sum per slot (+4/iter)
    s_ev = nc.alloc_semaphore("s_ev")      # ACT evict per slot (+4/iter)
    s_fin = nc.alloc_semaphore("s_fin")    # final ACT rescale (4)
    s_out = nc.alloc_semaphore("s_out")    # output DMA

    N_MEMSET = 6

    def gk(k):
        return k // 3, k % 3

    def emit_dve_gtmuls(v, s):
        v.wait_ge(s_hz, 14)   # gt pad memsets retired (same-engine WAW)
        v.wait_ge(s_rexp, s + 1)
        for k in DVE_SETUP_KS:
            kg_, kj_ = gk(k)
            dj = OFFS[k][1]
            v.tensor_tensor(
                out=gt.ap()[:, kg_, kj_, s, 1 + dj:513 + dj],
                in0=graw.ap()[:, CH_FOR_K[k], s, 1:513],
                in1=rr.ap()[:, s, 1:513],
                op=AluOpType.mult,
            ).then_inc(s_gtd, 1)

    def h_bcast(h, lo, hi, width):
        """h slots [lo:hi] broadcast over the 3-tap (kj) dim."""
        n = hi - lo
        return (h.ap()[:, lo:hi, 0:514]
                .unsqueeze(2).broadcast_to([width, n, 3, 514]))

    with nc.Block() as block:

        # ---------------- GPSIMD/Pool: DMAs, stationaries, products ----------
        @block.gpsimd
        def _(gp):
            # guidance slot 0 first -- needs no memsets, so it starts at t=0
            g_in0 = bass.AP(g_dram, 0, [[4 * W, 128], [H * W, 8], [1, W]])
            gp.dma_start(graw.ap()[:, :, 0, 1:513], g_in0).then_inc(s_gs[0], 16)
            gp.wait_ge(s_hz, 3)
            for w_t, base in ((ident, 0), (wup, 1), (wdn, -1)):
                gp.affine_select(
                    out=w_t.ap(), in_=w_t.ap(),
                    compare_op=AluOpType.not_equal, fill=1.0, base=base,
                    pattern=[[-1, 128]], channel_multiplier=1,
                ).then_inc(s_id, 1)
            for s in range(1, NS):
                g_in = bass.AP(g_dram, s * W,
                               [[4 * W, 128], [H * W, 8], [1, W]])
                gp.dma_start(graw.ap()[:, :, s, 1:513], g_in).then_inc(s_gs[s], 16)
            # x last: it is not needed until the first products (~cev0 time)
            gp.wait_ge(s_hz, 5)
            x_in = bass.AP(x_dram, 0, [[4 * W, 128], [W, NS], [1, W]])
            gp.dma_start(h0.ap()[:, :, 1:513], x_in).then_inc(s_x, 16)
            # iteration products: slot 2, one stt per tap-triple (kg) so each
            # starts as soon as its h slot (1+kg) is evicted
            def pool_kg(t, hcur, kg_):
                gp.tensor_tensor(
                    out=pr2.ap()[:, t % 2, kg_, :, 0:514],
                    in0=gt.ap()[:, kg_, :, 2, 0:514],
                    in1=(hcur.ap()[:, 1 + kg_:2 + kg_, 0:514]
                         .broadcast_to([128, 3, 514])),
                    op=AluOpType.mult,
                ).then_inc(s_mulp, 1)

            def pool_k6(t, hcur):
                gp.tensor_tensor(
                    out=pr2.ap()[:, t % 2, 2, 0, 0:514],
                    in0=gt.ap()[:, 2, 0, 2, 0:514],
                    in1=hcur.ap()[:, 3, 0:514],
                    op=AluOpType.mult,
                ).then_inc(s_mulp, 1)

            def pool_k78s(t, hcur):
                gp.tensor_tensor(
                    out=pr2.ap()[:, t % 2, 2, 1:3, 0:CSPL],
                    in0=gt.ap()[:, 2, 1:3, 2, 0:CSPL],
                    in1=(hcur.ap()[:, 3:4, 0:CSPL]
                         .broadcast_to([128, 2, CSPL])),
                    op=AluOpType.mult,
                ).then_inc(s_mulp, 1)

            # setup scale-muls for taps k=6,7 of each slot
            gp.wait_ge(s_hz, 14)   # gt pad memsets (WAW on shared columns)
            for s in range(NS):
                gp.wait_ge(s_rexp, s + 1)
                for k in POOL_SETUP_KS:
                    kg_, kj_ = gk(k)
                    dj = OFFS[k][1]
                    gp.tensor_tensor(
                        out=gt.ap()[:, kg_, kj_, s, 1 + dj:513 + dj],
                        in0=graw.ap()[:, CH_FOR_K[k], s, 1:513],
                        in1=rr.ap()[:, s, 1:513],
                        op=AluOpType.mult,
                    ).then_inc(s_gtp, 1)
            # t=0: kg1 holds the center tap (needs cev2); run it last
            gp.wait_ge(s_gtd, 18)
            gp.wait_ge(s_gtp, 8)   # own setup-mul writes retired
            gp.wait_ge(s_x, 16)
            pool_kg(0, h0, 0)
            pool_k6(0, h0)
            pool_k78s(0, h0)
            gp.wait_ge(s_cev, 3)
            pool_kg(0, h0, 1)
            for t in range(1, niter):
                hcur = hb[t % 2]
                gp.wait_ge(s_ev, 4 * (t - 1) + EVPOS[1])
                if t >= 2:
                    gp.wait_ge(s_pe, 6 * (t - 2) + 5)
                pool_kg(t, hcur, 0)
                gp.wait_ge(s_ev, 4 * (t - 1) + EVPOS[2])
                pool_kg(t, hcur, 1)
                gp.wait_ge(s_ev, 4 * (t - 1) + EVPOS[3])
                pool_k6(t, hcur)
                pool_k78s(t, hcur)
            gp.wait_ge(s_out, 64)

        # ---------------- DVE: memsets, abs, setup muls, products ------------
        @block.vector
        def _(v):
            v.memset(ident.ap(), 0.0).then_inc(s_hz, 1)
            v.memset(wup.ap(), 0.0).then_inc(s_hz, 1)
            v.memset(wdn.ap(), 0.0).then_inc(s_hz, 1)
            # h pad columns only; bodies are written by the x DMA (h0) and
            # by evictions (h1) before any read
            v.memset(h0.ap()[:, :, 0:1], 0.0).then_inc(s_hz, 1)
            v.memset(h0.ap()[:, :, 513:520], 0.0).then_inc(s_hz, 1)
            v.memset(c_eps.ap(), 1e-8).then_inc(s_hz, 1)
            v.memset(h1.ap()[:, :, 0:1], 0.0).then_inc(s_hz, 1)
            v.memset(h1.ap()[:, :, 513:520], 0.0).then_inc(s_hz, 1)
            # abs per slot as guidance DMAs land; pad memsets fill the gaps
            # (their consumers are transitively gated through s_gtd/s_mul)
            for s in range(NS):
                v.wait_ge(s_gs[s], 16)
                if s >= 2:
                    v.wait_ge(s_apex, s - 1)  # gabs[s%2] free again
                v.tensor_scalar(
                    out=gabs.ap()[:, s % 2, :, 1:513].bitcast(U16),
                    in0=graw.ap()[:, :, s, 1:513].bitcast(U16),
                    scalar1=0x7FFF, scalar2=None, op0=AluOpType.bitwise_and,
                ).then_inc(s_abs, 1)
                if s == 0:
                    for kg_ in range(3):
                        v.memset(gt.ap()[:, kg_, :, :, 0:2], 0.0).then_inc(s_hz, 1)
                        v.memset(gt.ap()[:, kg_, :, :, 512:520], 0.0).then_inc(s_hz, 1)
                else:
                    # slot s-1 scale-muls, interleaved so they start as soon
                    # as exp(s-1) lands instead of after all four abs passes
                    emit_dve_gtmuls(v, s - 1)
            emit_dve_gtmuls(v, 3)
            # iteration products: 7 instructions per iteration, ordered by
            # gate availability then consumer deadline.  PE window order:
            # 0a'(k678), slot1, 0a''(k345), 0b(cross), slot2, slot3.
            # t=0 uses a different order and precise setup gates (only the
            # tap-groups containing the center k=4 need the center-evict).
            def p_s0kg2(t, hcur):
                v.tensor_tensor(
                    out=pr.ap()[:, 2, :, 0, 0:514],
                    in0=gt.ap()[:, 2, :, 0, 0:514],
                    in1=(hcur.ap()[:, 1:2, 0:514]
                         .broadcast_to([128, 3, 514])),
                    op=AluOpType.mult,
                ).then_inc(s_mul, 1)

            def p_s1kg01(t, hcur):
                v.tensor_tensor(
                    out=pr.ap()[:, 0:2, :, 1, 0:514],
                    in0=gt.ap()[:, 0:2, :, 1, 0:514],
                    in1=h_bcast(hcur, 0, 2, 128), op=AluOpType.mult,
                ).then_inc(s_mul, 1)

            def p_s1kg2(t, hcur):
                v.tensor_tensor(
                    out=pr.ap()[:, 2, :, 1, 0:514],
                    in0=gt.ap()[:, 2, :, 1, 0:514],
                    in1=(hcur.ap()[:, 2:3, 0:514]
                         .broadcast_to([128, 3, 514])),
                    op=AluOpType.mult,
                ).then_inc(s_mul, 1)

            def p_s0kg1(t, hcur):
                v.tensor_tensor(
                    out=pr.ap()[:, 1, :, 0, 0:514],
                    in0=gt.ap()[:, 1, :, 0, 0:514],
                    in1=(hcur.ap()[:, 0:1, 0:514]
                         .broadcast_to([128, 3, 514])),
                    op=AluOpType.mult,
                ).then_inc(s_mul, 1)

            def p_up3(t, hcur):
                v.tensor_tensor(
                    out=prc.ap()[:, 0:3, 0:514],
                    in0=gtc.ap()[:, 0:3, 0:514],
                    in1=(hcur.ap()[:, 3:4, 0:514]
                         .broadcast_to([128, 3, 514])),
                    op=AluOpType.mult,
                ).then_inc(s_mul, 1)

            def p_s3kg01(t, hcur):
                v.tensor_tensor(
                    out=pr.ap()[:, 0:2, :, 3, 0:514],
                    in0=gt.ap()[:, 0:2, :, 3, 0:514],
                    in1=h_bcast(hcur, 2, 4, 128), op=AluOpType.mult,
                ).then_inc(s_mul, 1)

            def p_s2k78(t, hcur):
                # cols [CSPL:514]; Pool covers [0:CSPL] of the same rows
                v.tensor_tensor(
                    out=pr2.ap()[:, t % 2, 2, 1:3, CSPL:514],
                    in0=gt.ap()[:, 2, 1:3, 2, CSPL:514],
                    in1=(hcur.ap()[:, 3:4, CSPL:514]
                         .broadcast_to([128, 2, 514 - CSPL])),
                    op=AluOpType.mult,
                ).then_inc(s_mul, 1)

            def p_dn3(t, hcur):
                v.tensor_tensor(
                    out=prc.ap()[:, 3:6, 0:514],
                    in0=gtc.ap()[:, 3:6, 0:514],
                    in1=(hcur.ap()[:, 0:1, 0:514]
                         .broadcast_to([128, 3, 514])),
                    op=AluOpType.mult,
                ).then_inc(s_mul, 1)

            # t=0: split the center-bearing tap-groups so only three tail
            # instructions wait on center-evicts; everything else is gated by
            # the scale-muls / gtc shifts alone
            def p_kg(sg, kg_, hslot):
                v.tensor_tensor(
                    out=pr.ap()[:, kg_, :, sg, 0:514],
                    in0=gt.ap()[:, kg_, :, sg, 0:514],
                    in1=(h0.ap()[:, hslot:hslot + 1, 0:514]
                         .broadcast_to([128, 3, 514])),
                    op=AluOpType.mult,
                ).then_inc(s_mul, 1)

            v.wait_ge(s_gtd, 6)
            v.wait_ge(s_gtp, 2)
            v.wait_ge(s_x, 16)
            p_s0kg2(0, h0)            # 1
            v.wait_ge(s_gtd, 12)
            v.wait_ge(s_gtp, 4)
            p_kg(1, 0, 0)             # 2: slot1 k0-2
            p_s1kg2(0, h0)            # 3
            v.wait_ge(s_gtcu, 32)
            p_up3(0, h0)              # 4
            v.wait_ge(s_gtcd, 32)
            p_dn3(0, h0)              # 5
            v.wait_ge(s_gtd, 24)
            p_kg(3, 0, 2)             # 6: slot3 k0-2
            v.wait_ge(s_gtp, 6)
            p_s2k78(0, h0)            # 7: slot2 k7,k8
            v.wait_ge(s_cev, 1)
            p_kg(0, 1, 0)             # 8: slot0 k3-5 (center)
            v.wait_ge(s_cev, 2)
            p_kg(1, 1, 1)             # 9: slot1 k3-5 (center)
            v.wait_ge(s_cev, 4)
            p_kg(3, 1, 3)             # 10: slot3 k3-5 (center)
            # t>=1: steady-state order and gates
            for t in range(1, niter):
                hcur = hb[t % 2]
                v.wait_ge(s_ev, 4 * (t - 1) + 1)
                v.wait_ge(s_pe, 6 * (t - 1) + 1)
                p_s0kg2(t, hcur)
                v.wait_ge(s_ev, 4 * (t - 1) + 2)
                v.wait_ge(s_pe, 6 * (t - 1) + 2)
                p_s1kg01(t, hcur)
                v.wait_ge(s_ev, 4 * (t - 1) + 3)
                p_s1kg2(t, hcur)
                v.wait_ge(s_pe, 6 * (t - 1) + 3)
                p_s0kg1(t, hcur)
                v.wait_ge(s_ev, 4 * (t - 1) + 4)
                v.wait_ge(s_pe, 6 * (t - 1) + 4)
                p_up3(t, hcur)
                if t >= 2:
                    v.wait_ge(s_pe, 6 * (t - 2) + 5)   # pr2 parity WAR
                p_s2k78(t, hcur)
                v.wait_ge(s_pe, 6 * (t - 1) + 6)
                p_s3kg01(t, hcur)
                p_dn3(t, hcur)

        # ---------------- PE: setup sums + iteration tap-sums ----------------
        @block.tensor
        def _(pe):
            pe.wait_ge(s_id, 3)
            # warm-up: ramp the PE p-state past the 3us threshold on dummy
            # 128-col matmuls so the first A-sum runs at full clock
            for _ in range(30):
                pe.matmul(pg(1, 0)[:, 0:128], ident.ap(), ident.ap(),
                          start=True, stop=True, skip_group_check=True)
            # A = sum_ch |g_ch| into psum group 0
            for s in range(NS):
                pe.wait_ge(s_abs, s + 1)
                for ch in range(8):
                    inst = pe.matmul(
                        pg(0, s)[:, 0:512], ident.ap(),
                        gabs.ap()[:, s % 2, ch, 1:513],
                        start=(ch == 0), stop=(ch == 7),
                    )
                    if ch == 7:
                        inst.then_inc(s_apex, 1)
            # center gate: sum of fp16-rounded weights into psum group 1
            for s in range(NS):
                pe.wait_ge(s_gtd, len(DVE_SETUP_KS) * (s + 1))
                pe.wait_ge(s_gtp, len(POOL_SETUP_KS) * (s + 1))
                done = 0
                for k in range(9):
                    if k == 4:
                        continue
                    kg_, kj_ = gk(k)
                    dj = OFFS[k][1]
                    inst = pe.matmul(
                        pg(1, s)[:, 0:512], ident.ap(),
                        gt.ap()[:, kg_, kj_, s, 1 + dj:513 + dj],
                        start=(done == 0), stop=(done == 7),
                    )
                    done += 1
                    if done == 8:
                        inst.then_inc(s_cpe, 1)
            # keepalive: bridge the idle gap between setup and the first
            # window so the p-state ramp survives into T0's matmuls.
            # pg(0,0)'s only setup reader (ln0) is gated done via s_ln.
            pe.wait_ge(s_ln, 1)
            for _ in range(N_KEEPALIVE):
                pe.matmul(pg(0, 0)[:, 0:128], ident.ap(), ident.ap(),
                          start=True, stop=True, skip_group_check=True)
            # iterations: window order 0a'(k678, start psum0), slot1,
            # 0a''(k345), 0b(cross, stop psum0), slot2, slot3;
            # s_pe +1 after each of the 6 groups
            for t in range(niter):
                # t=0 DVE emission order differs; map each group to the
                # s_mul count of its last-needed product
                mw = ([1, 9, 8, 4, 7, 10] if t == 0
                      else [N_T0_MUL + 8 * (t - 1) + 1, N_T0_MUL + 8 * (t - 1) + 3,
                            N_T0_MUL + 8 * (t - 1) + 4, N_T0_MUL + 8 * (t - 1) + 5,
                            N_T0_MUL + 8 * (t - 1) + 6, N_T0_MUL + 8 * (t - 1) + 8])
                groups = [
                    # (psum slot, taps, s_mul wait, s_mulp wait, start, stop)
                    (0, [(k, ident, None) for k in range(6, 9)],
                     mw[0], None, True, False),
                    (1, [(k, ident, None) for k in range(9)],
                     mw[1], None, True, True),
                    (0, [(k, ident, None) for k in range(3, 6)],
                     mw[2], None, False, False),
                    (0, [(k, wup, k) for k in range(3)],
                     mw[3], None, False, True),
                    (2, [(k, ident, None) for k in range(9)],
                     mw[4], 4 * t + 4, True, True),
                    (3, [(k, ident, None) for k in range(6)]
                        + [(k, wdn, 3 + k - 6) for k in range(6, 9)],
                     mw[5], None, True, True),
                ]
                for sg, taps, mw, mpw, st, sp in groups:
                    if mw is not None:
                        pe.wait_ge(s_mul, mw)
                    if mpw is not None:
                        pe.wait_ge(s_mulp, mpw)
                    if st:
                        # psum(sg, t%2) must be drained by its previous reader
                        if t == 0:
                            pe.wait_ge(s_ln, sg + 1)
                        elif t == 1:
                            pe.wait_ge(s_cev, sg + 1)
                        else:
                            pe.wait_ge(s_ev, 4 * (t - 2) + EVPOS[sg])
                    n = len(taps)
                    for j, (k, w_t, crow) in enumerate(taps):
                        kg_, kj_ = gk(k)
                        dj = OFFS[k][1]
                        if sg == 2:
                            mv = pr2.ap()[:, t % 2, kg_, kj_, 1 + dj:513 + dj]
                        elif crow is None:
                            mv = pr.ap()[:, kg_, kj_, sg, 1 + dj:513 + dj]
                        else:
                            mv = prc.ap()[:, crow, 1 + dj:513 + dj]
                        inst = pe.matmul(
                            pg(t % 2, sg)[:, 0:512], w_t.ap(), mv,
                            start=(st and j == 0), stop=(sp and j == n - 1),
                            skip_group_check=True,
                        )
                        if j == n - 1:
                            inst.then_inc(s_pe, 1)

        # ---------------- ACT: ln/exp normalization, evictions ---------------
        @block.scalar
        def _(sc):
            sc.wait_ge(s_hz, 6)   # c_eps ready
            for s in range(NS):
                sc.wait_ge(s_apex, s + 1)
                sc.activation(
                    asb.ap()[:, s, 1:513], pg(0, s)[:, 0:512], AF.Ln,
                    bias=c_eps.ap(),
                ).then_inc(s_ln, 1)
                sc.wait_ge(s_ln, s + 1)
                sc.activation(
                    rr.ap()[:, s, 1:513], asb.ap()[:, s, 1:513], AF.Exp,
                    scale=-1.0,
                ).then_inc(s_rexp, 1)
            for s in range(NS):
                sc.wait_ge(s_cpe, s + 1)
                sc.activation(
                    gt.ap()[:, 1, 1, s, 1:513], pg(1, s)[:, 0:512], AF.Identity,
                    bias=1.0, scale=-1.0,
                ).then_inc(s_cev, 1)
            # eviction order [1, 0, 2, 3] matches PE stop-group order
            pe_stop = {1: 2, 0: 4, 2: 5, 3: 6}
            # DVE cycle-(t-1) readers of h slot sg (last reading instr index)
            war_mul = {0: 8, 1: 2, 2: 7, 3: 7}
            war_mulp = {1: 1, 2: 2, 3: 4}
            for t in range(niter):
                hnext = hb[(t + 1) % 2]
                last = (t == niter - 1)
                for sg in EVORD:
                    sc.wait_ge(s_pe, 6 * t + pe_stop[sg])
                    if t == 0 and sg == EVORD[0]:
                        sc.wait_ge(s_hz, 8)   # h1 pad memsets done
                    if t > 0:
                        # h_next WAR: cycle t-1 readers of this buffer+slot
                        sc.wait_ge(s_mul, N_T0_MUL if t == 1
                                   else N_T0_MUL + 8 * (t - 2) + war_mul[sg])
                        if sg in war_mulp:
                            sc.wait_ge(s_mulp, 4 * (t - 1) + war_mulp[sg])
                    if last:
                        # final eviction straight to fp32 at true magnitude
                        # (undoes the 23 earlier rescales plus this one)
                        sc.activation(
                            asb.ap()[:, sg, 1:513],
                            pg(t % 2, sg)[:, 0:512], AF.Copy,
                            scale=RESCALE * float(1.0 / RESCALE) ** niter,
                        ).then_inc(s_fin, 1)
                    else:
                        sc.activation(
                            hnext.ap()[:, sg, 1:513],
                            pg(t % 2, sg)[:, 0:512], AF.Copy,
                            scale=RESCALE,
                        ).then_inc(s_ev, 1)

        # ---------------- SYNC: gtc shift DMAs + output ----------------------
        @block.sync
        def _(sy):
            # cross weights: one-time partition shifts of the normalized gt
            sy.wait_ge(s_gtd, 3)
            sy.dma_start(gtc.ap()[0:127, 0:3, 0:514],
                         gt.ap()[1:128, 0, :, 0, 0:514]).then_inc(s_gtcu, 16)
            sy.dma_start(gtc.ap()[127:128, 0:3, 0:514],
                         gt.ap()[0:1, 0, :, 0, 0:514]).then_inc(s_gtcu, 16)
            sy.wait_ge(s_gtd, len(DVE_SETUP_KS) * NS)
            sy.wait_ge(s_gtp, len(POOL_SETUP_KS) * NS)
            sy.dma_start(gtc.ap()[1:128, 3:6, 0:514],
                         gt.ap()[0:127, 2, :, 3, 0:514]).then_inc(s_gtcd, 16)
            sy.dma_start(gtc.ap()[0:1, 3:6, 0:514],
                         gt.ap()[127:128, 2, :, 3, 0:514]).then_inc(s_gtcd, 16)
            # per-slot output DMAs, each as soon as its final eviction lands
            for i, sg in enumerate(EVORD):
                sy.wait_ge(s_fin, i + 1)
                o_out = bass.AP(o_dram, sg * W, [[4 * W, 128], [1, W]])
                sy.dma_start(o_out, asb.ap()[:, sg, 1:513]).then_inc(s_out, 16)

    return nc


_NC_CACHE = {}


def kernel(guidance: np.ndarray, x: np.ndarray) -> np.ndarray:
    """guidance [8,8,512,512] f32, x [8,1,512,512] f32 -> [8,1,512,512] f32."""
    guidance = np.ascontiguousarray(np.asarray(guidance, dtype=np.float32))
    x = np.ascontiguousarray(np.asarray(x, dtype=np.float32))
    if "nc" not in _NC_CACHE:
        _NC_CACHE["nc"] = build_program()
    nc = _NC_CACHE["nc"]
    in_maps = [
        {"guidance": guidance[b], "x": x[b].reshape(1, H, W)}
        for b in range(N_CORES)
    ]
    res = run_bass_kernel_spmd(nc, in_maps, core_ids=list(range(N_CORES)))
    out = np.stack([res.results[b]["out"] for b in range(N_CORES)], axis=0)
    return out.astype(np.float32)


# revision 9
# speedup vs baseline: 1.0209x; 1.0001x over previous
"""CSPN (convolutional spatial propagation) Trainium2 kernel, v2.

Full inputs:  guidance [8, 8, 512, 512] f32, x [8, 1, 512, 512] f32.
Sharding: data-parallel over batch -- core b gets batch element b.

v2 redesign vs the halo-DMA baseline:
  * No halo rows at all.  h is [128, 4, PW]: partition p holds image rows
    4p..4p+3 (slots 0-3).  The 6 tap-rows whose input row lives on a
    neighbouring partition (slot0 di=-1, slot3 di=+1) are multiplied ON the
    partition that owns the h row, against one-time partition-shifted copies
    of the weights (gtc).  The PE re-aligns those products to the output
    partition with shifted-identity stationary matrices (free in both the
    cost model and -- via dual weight buffers -- nearly free on HW).
  * Product rows are split DVE 29 / Pool 7 per iteration (the real
    TRN2 ISA allows only plain tensor_tensor on Pool -- the faster
    TensorScalarPtr family is DVE-only).  Setup scale-muls all run
    on DVE in 2x mode (rr stored f16); PE p-state is kept warm with
    dummy-matmul keepalive blocks.
  * DVE products are merged into 5 wide instructions per iteration
    ([2-3, 3, 514] APs with double-broadcast h), cutting per-instruction
    overhead; Pool does one [3, 3, 514] instruction (slot 2).
  * Steady state is PE-bound at 36 matmul-columns x 512 per iteration
    (~7.7 us); DVE ~7.5 us, Pool ~6.5 us, ACT ~2.5 us.

Numerics are identical to the baseline: fp16 weights/products, fp32 PSUM
accumulation, center gate gt4 = 1 - sum(fp16-rounded gt_k), h rescaled by
0.5/iter to stay in fp16 range, un-scaled by 2^24 in the final fp32 pass.
"""

import sys

sys.path.insert(0, "/opt/trn_rl_repo")

import numpy as np

import concourse.bass as bass
from concourse import mybir
from concourse.bass_utils import run_bass_kernel_spmd
from concourse.alu_op_type import AluOpType

F16 = mybir.dt.float16
F32 = mybir.dt.float32
U16 = mybir.dt.uint16
AF = mybir.ActivationFunctionType

N_CORES = 8
H, W = 512, 512
NS = 4            # row-slots per partition
PW = 520          # f16 elements per row slot (514 used + pad)
NITER = 24
RESCALE = 0.5
# tap k = (di+1)*3 + (dj+1); k=4 is the center gate
OFFS = [(k // 3 - 1, k % 3 - 1) for k in range(9)]
# guidance channel for tap k (center has none)
CH_FOR_K = [0, 1, 2, 3, None, 4, 5, 6, 7]
# setup scale-muls: 6 on DVE (rr f16 -> 2x mode), 2 on the otherwise-idle
# Pool via plain tensor_tensor (the only mult op the real ISA allows there)
DVE_SETUP_KS = (0, 1, 2, 3, 5, 8)
POOL_SETUP_KS = (6, 7)
# eviction order within a cycle (follows the PE stop-group order) and the
# 1-based position of each slot's eviction in that order
EVORD = [1, 0, 2, 3]
EVPOS = {1: 1, 0: 2, 2: 3, 3: 4}
# DVE product-instruction count at t=0 (9: split center groups)
N_T0_MUL = 10
N_KEEPALIVE = 60
# column split of the slot-2 k7/k8 rows between Pool [0:CSPL] and DVE
CSPL = 127


def build_program(niter=NITER):
    nc = bass.Bass("TRN2", target_bir_lowering=False, debug=False)

    g_dram = nc.dram_tensor("guidance", [8, H, W], F32, kind="ExternalInput")
    x_dram = nc.dram_tensor("x", [1, H, W], F32, kind="ExternalInput")
    o_dram = nc.dram_tensor("out", [1, H, W], F32, kind="ExternalOutput")

    h0 = nc.alloc_sbuf_tensor("h0", [128, NS, PW], F16)
    h1 = nc.alloc_sbuf_tensor("h1", [128, NS, PW], F16)
    # weights / products indexed [kg, kj, slot, col] with tap k = 3*kg + kj
    gt = nc.alloc_sbuf_tensor("gt", [128, 3, 3, NS, PW], F16)
    pr = nc.alloc_sbuf_tensor("pr", [128, 3, 3, NS, PW], F16)
    # slot-2 products, double-buffered so Pool(t+1) never waits on PE2(t)
    pr2 = nc.alloc_sbuf_tensor("pr2", [128, 2, 3, 3, PW], F16)
    # cross rows: 0-2 = slot0 taps k=0,1,2 shifted up (gtc[p] = gt[p+1]);
    #             3-5 = slot3 taps k=6,7,8 shifted down (gtc[p] = gt[p-1])
    gtc = nc.alloc_sbuf_tensor("gtc", [128, 6, PW], F16)
    prc = nc.alloc_sbuf_tensor("prc", [128, 6, PW], F16)
    graw = nc.alloc_sbuf_tensor("graw", [128, 8, NS, PW], F16)
    gabs = nc.alloc_sbuf_tensor("gabs", [128, 2, 8, PW], F16)
    asb = nc.alloc_sbuf_tensor("asb", [128, NS, PW], F32)   # ln(A); out stage
    rr = nc.alloc_sbuf_tensor("rr", [128, NS, PW], F16)     # r = 1/A
    ident = nc.alloc_sbuf_tensor("ident", [128, 128], F16)
    wup = nc.alloc_sbuf_tensor("wup", [128, 128], F16)      # psum[p] += x[p-1]
    wdn = nc.alloc_sbuf_tensor("wdn", [128, 128], F16)      # psum[p] += x[p+1]
    c_eps = nc.alloc_sbuf_tensor("c_eps", [128, 1], F32)

    psum = [nc.alloc_psum_tensor(f"pg{g}s{s}", [128, W], F32)
            for g in range(2) for s in range(NS)]

    def pg(g, s):
        return psum[g * NS + s].ap()

    hb = [h0, h1]

    s_hz = nc.alloc_semaphore("s_hz")      # DVE memsets done
    s_id = nc.alloc_semaphore("s_id")      # stationaries built (3)
    s_x = nc.alloc_semaphore("s_x")        # x DMA (+16)
    s_gs = [nc.alloc_semaphore(f"s_g{i}") for i in range(NS)]
    s_abs = nc.alloc_semaphore("s_abs")    # DVE abs per slot
    s_apex = nc.alloc_semaphore("s_apex")  # PE A-sum per slot
    s_ln = nc.alloc_semaphore("s_ln")      # ACT ln per slot
    s_rexp = nc.alloc_semaphore("s_rexp")  # ACT exp per slot
    s_gtd = nc.alloc_semaphore("s_gtd")    # DVE setup muls (5/slot)
    s_gtp = nc.alloc_semaphore("s_gtp")    # Pool setup muls (3/slot)
    s_cpe = nc.alloc_semaphore("s_cpe")    # PE center sum per slot
    s_cev = nc.alloc_semaphore("s_cev")    # ACT center evict per slot
    s_gtcu = nc.alloc_semaphore("s_gtcu")  # gtc up-shift DMA (+16)
    s_gtcd = nc.alloc_semaphore("s_gtcd")  # gtc down-shift DMA (+16)
    s_mul = nc.alloc_semaphore("s_mul")    # DVE iter muls (+5/iter)
    s_mulp = nc.alloc_semaphore("s_mulp")  # Pool iter muls (+1/iter)
    s_pe = nc.alloc_semaphore("s_pe")      # PE tap-sum per slot (+4/iter)
    s_ev = nc.alloc_semaphore("s_ev")      # ACT evict per slot (+4/iter)
    s_fin = nc.alloc_semaphore("s_fin")    # final ACT rescale (4)
    s_out = nc.alloc_semaphore("s_out")    # output DMA

    N_MEMSET = 6

    def gk(k):
        return k // 3, k % 3

    def emit_dve_gtmuls(v, s):
        v.wait_ge(s_hz, 14)   # gt pad memsets retired (same-engine WAW)
        v.wait_ge(s_rexp, s + 1)
        for k in DVE_SETUP_KS:
            kg_, kj_ = gk(k)
            dj = OFFS[k][1]
            v.tensor_tensor(
                out=gt.ap()[:, kg_, kj_, s, 1 + dj:513 + dj],
                in0=graw.ap()[:, CH_FOR_K[k], s, 1:513],
                in1=rr.ap()[:, s, 1:513],
                op=AluOpType.mult,
            ).then_inc(s_gtd, 1)

    def h_bcast(h, lo, hi, width):
        """h slots [lo:hi] broadcast over the 3-tap (kj) dim."""
        n = hi - lo
        return (h.ap()[:, lo:hi, 0:514]
                .unsqueeze(2).broadcast_to([width, n, 3, 514]))

    with nc.Block() as block:

        # ---------------- GPSIMD/Pool: DMAs, stationaries, products ----------
        @block.gpsimd
        def _(gp):
            # guidance slot 0 first -- needs no memsets, so it starts at t=0
            g_in0 = bass.AP(g_dram, 0, [[4 * W, 128], [H * W, 8], [1, W]])
            gp.dma_start(graw.ap()[:, :, 0, 1:513], g_in0).then_inc(s_gs[0], 16)
            gp.wait_ge(s_hz, 3)
            for w_t, base in ((ident, 0), (wup, 1), (wdn, -1)):
                gp.affine_select(
                    out=w_t.ap(), in_=w_t.ap(),
                    compare_op=AluOpType.not_equal, fill=1.0, base=base,
                    pattern=[[-1, 128]], channel_multiplier=1,
                ).then_inc(s_id, 1)
            for s in range(1, NS):
                g_in = bass.AP(g_dram, s * W,
                               [[4 * W, 128], [H * W, 8], [1, W]])
                gp.dma_start(graw.ap()[:, :, s, 1:513], g_in).then_inc(s_gs[s], 16)
            # x last: it is not needed until the first products (~cev0 time)
            gp.wait_ge(s_hz, 5)
            x_in = bass.AP(x_dram, 0, [[4 * W, 128], [W, NS], [1, W]])
            gp.dma_start(h0.ap()[:, :, 1:513], x_in).then_inc(s_x, 16)
            # iteration products: slot 2, one stt per tap-triple (kg) so each
            # starts as soon as its h slot (1+kg) is evicted
            def pool_kg(t, hcur, kg_):
                gp.tensor_tensor(
                    out=pr2.ap()[:, t % 2, kg_, :, 0:514],
                    in0=gt.ap()[:, kg_, :, 2, 0:514],
                    in1=(hcur.ap()[:, 1 + kg_:2 + kg_, 0:514]
                         .broadcast_to([128, 3, 514])),
                    op=AluOpType.mult,
                ).then_inc(s_mulp, 1)

            def pool_k6(t, hcur):
                gp.tensor_tensor(
                    out=pr2.ap()[:, t % 2, 2, 0, 0:514],
                    in0=gt.ap()[:, 2, 0, 2, 0:514],
                    in1=hcur.ap()[:, 3, 0:514],
                    op=AluOpType.mult,
                ).then_inc(s_mulp, 1)

            def pool_k78s(t, hcur):
                gp.tensor_tensor(
                    out=pr2.ap()[:, t % 2, 2, 1:3, 0:CSPL],
                    in0=gt.ap()[:, 2, 1:3, 2, 0:CSPL],
                    in1=(hcur.ap()[:, 3:4, 0:CSPL]
                         .broadcast_to([128, 2, CSPL])),
                    op=AluOpType.mult,
                ).then_inc(s_mulp, 1)

            # setup scale-muls for taps k=6,7 of each slot
            gp.wait_ge(s_hz, 14)   # gt pad memsets (WAW on shared columns)
            for s in range(NS):
                gp.wait_ge(s_rexp, s + 1)
                for k in POOL_SETUP_KS:
                    kg_, kj_ = gk(k)
                    dj = OFFS[k][1]
                    gp.tensor_tensor(
                        out=gt.ap()[:, kg_, kj_, s, 1 + dj:513 + dj],
                        in0=graw.ap()[:, CH_FOR_K[k], s, 1:513],
                        in1=rr.ap()[:, s, 1:513],
                        op=AluOpType.mult,
                    ).then_inc(s_gtp, 1)
            # t=0: kg1 holds the center tap (needs cev2); run it last
            gp.wait_ge(s_gtd, 18)
            gp.wait_ge(s_gtp, 8)   # own setup-mul writes retired
            gp.wait_ge(s_x, 16)
            pool_kg(0, h0, 0)
            pool_k6(0, h0)
            pool_k78s(0, h0)
            gp.wait_ge(s_cev, 3)
            pool_kg(0, h0, 1)
            for t in range(1, niter):
                hcur = hb[t % 2]
                gp.wait_ge(s_ev, 4 * (t - 1) + EVPOS[1])
                if t >= 2:
                    gp.wait_ge(s_pe, 6 * (t - 2) + 5)
                pool_kg(t, hcur, 0)
                gp.wait_ge(s_ev, 4 * (t - 1) + EVPOS[2])
                pool_kg(t, hcur, 1)
                gp.wait_ge(s_ev, 4 * (t - 1) + EVPOS[3])
                pool_k6(t, hcur)
                pool_k78s(t, hcur)
            gp.wait_ge(s_out, 64)

        # ---------------- DVE: memsets, abs, setup muls, products ------------
        @block.vector
        def _(v):
            v.memset(ident.ap(), 0.0).then_inc(s_hz, 1)
            v.memset(wup.ap(), 0.0).then_inc(s_hz, 1)
            v.memset(wdn.ap(), 0.0).then_inc(s_hz, 1)
            # h pad columns only; bodies are written by the x DMA (h0) and
            # by evictions (h1) before any read
            v.memset(h0.ap()[:, :, 0:1], 0.0).then_inc(s_hz, 1)
            v.memset(h0.ap()[:, :, 513:520], 0.0).then_inc(s_hz, 1)
            v.memset(c_eps.ap(), 1e-8).then_inc(s_hz, 1)
            v.memset(h1.ap()[:, :, 0:1], 0.0).then_inc(s_hz, 1)
            v.memset(h1.ap()[:, :, 513:520], 0.0).then_inc(s_hz, 1)
            # abs per slot as guidance DMAs land; pad memsets fill the gaps
            # (their consumers are transitively gated through s_gtd/s_mul)
            for s in range(NS):
                v.wait_ge(s_gs[s], 16)
                if s >= 2:
                    v.wait_ge(s_apex, s - 1)  # gabs[s%2] free again
                v.tensor_scalar(
                    out=gabs.ap()[:, s % 2, :, 1:513].bitcast(U16),
                    in0=graw.ap()[:, :, s, 1:513].bitcast(U16),
                    scalar1=0x7FFF, scalar2=None, op0=AluOpType.bitwise_and,
                ).then_inc(s_abs, 1)
                if s == 0:
                    for kg_ in range(3):
                        v.memset(gt.ap()[:, kg_, :, :, 0:2], 0.0).then_inc(s_hz, 1)
                        v.memset(gt.ap()[:, kg_, :, :, 512:520], 0.0).then_inc(s_hz, 1)
                else:
                    # slot s-1 scale-muls, interleaved so they start as soon
                    # as exp(s-1) lands instead of after all four abs passes
                    emit_dve_gtmuls(v, s - 1)
            emit_dve_gtmuls(v, 3)
            # iteration products: 7 instructions per iteration, ordered by
            # gate availability then consumer deadline.  PE window order:
            # 0a'(k678), slot1, 0a''(k345), 0b(cross), slot2, slot3.
            # t=0 uses a different order and precise setup gates (only the
            # tap-groups containing the center k=4 need the center-evict).
            def p_s0kg2(t, hcur):
                v.tensor_tensor(
                    out=pr.ap()[:, 2, :, 0, 0:514],
                    in0=gt.ap()[:, 2, :, 0, 0:514],
                    in1=(hcur.ap()[:, 1:2, 0:514]
                         .broadcast_to([128, 3, 514])),
                    op=AluOpType.mult,
                ).then_inc(s_mul, 1)

            def p_s1kg01(t, hcur):
                v.tensor_tensor(
                    out=pr.ap()[:, 0:2, :, 1, 0:514],
                    in0=gt.ap()[:, 0:2, :, 1, 0:514],
                    in1=h_bcast(hcur, 0, 2, 128), op=AluOpType.mult,
                ).then_inc(s_mul, 1)

            def p_s1kg2(t, hcur):
                v.tensor_tensor(
                    out=pr.ap()[:, 2, :, 1, 0:514],
                    in0=gt.ap()[:, 2, :, 1, 0:514],
                    in1=(hcur.ap()[:, 2:3, 0:514]
                         .broadcast_to([128, 3, 514])),
                    op=AluOpType.mult,
                ).then_inc(s_mul, 1)

            def p_s0kg1(t, hcur):
                v.tensor_tensor(
                    out=pr.ap()[:, 1, :, 0, 0:514],
                    in0=gt.ap()[:, 1, :, 0, 0:514],
                    in1=(hcur.ap()[:, 0:1, 0:514]
                         .broadcast_to([128, 3, 514])),
                    op=AluOpType.mult,
                ).then_inc(s_mul, 1)

            def p_up3(t, hcur):
                v.tensor_tensor(
                    out=prc.ap()[:, 0:3, 0:514],
                    in0=gtc.ap()[:, 0:3, 0:514],
                    in1=(hcur.ap()[:, 3:4, 0:514]
                         .broadcast_to([128, 3, 514])),
                    op=AluOpType.mult,
                ).then_inc(s_mul, 1)

            def p_s3kg01(t, hcur):
                v.tensor_tensor(
                    out=pr.ap()[:, 0:2, :, 3, 0:514],
                    in0=gt.ap()[:, 0:2, :, 3, 0:514],
                    in1=h_bcast(hcur, 2, 4, 128), op=AluOpType.mult,
                ).then_inc(s_mul, 1)

            def p_s2k78(t, hcur):
                # cols [CSPL:514]; Pool covers [0:CSPL] of the same rows
                v.tensor_tensor(
                    out=pr2.ap()[:, t % 2, 2, 1:3, CSPL:514],
                    in0=gt.ap()[:, 2, 1:3, 2, CSPL:514],
                    in1=(hcur.ap()[:, 3:4, CSPL:514]
                         .broadcast_to([128, 2, 514 - CSPL])),
                    op=AluOpType.mult,
                ).then_inc(s_mul, 1)

            def p_dn3(t, hcur):
                v.tensor_tensor(
                    out=prc.ap()[:, 3:6, 0:514],
                    in0=gtc.ap()[:, 3:6, 0:514],
                    in1=(hcur.ap()[:, 0:1, 0:514]
                         .broadcast_to([128, 3, 514])),
                    op=AluOpType.mult,
                ).then_inc(s_mul, 1)

            # t=0: split the center-bearing tap-groups so only three tail
            # instructions wait on center-evicts; everything else is gated by
            # the scale-muls / gtc shifts alone
            def p_kg(sg, kg_, hslot):
                v.tensor_tensor(
                    out=pr.ap()[:, kg_, :, sg, 0:514],
                    in0=gt.ap()[:, kg_, :, sg, 0:514],
                    in1=(h0.ap()[:, hslot:hslot + 1, 0:514]
                         .broadcast_to([128, 3, 514])),
                    op=AluOpType.mult,
                ).then_inc(s_mul, 1)

            v.wait_ge(s_gtd, 6)
            v.wait_ge(s_gtp, 2)
            v.wait_ge(s_x, 16)
            p_s0kg2(0, h0)            # 1
            v.wait_ge(s_gtd, 12)
            v.wait_ge(s_gtp, 4)
            p_kg(1, 0, 0)             # 2: slot1 k0-2
            p_s1kg2(0, h0)            # 3
            v.wait_ge(s_gtcu, 32)
            p_up3(0, h0)              # 4
            v.wait_ge(s_gtcd, 32)
            p_dn3(0, h0)              # 5
            v.wait_ge(s_gtd, 24)
            p_kg(3, 0, 2)             # 6: slot3 k0-2
            v.wait_ge(s_gtp, 6)
            p_s2k78(0, h0)            # 7: slot2 k7,k8
            v.wait_ge(s_cev, 1)
            p_kg(0, 1, 0)             # 8: slot0 k3-5 (center)
            v.wait_ge(s_cev, 2)
            p_kg(1, 1, 1)             # 9: slot1 k3-5 (center)
            v.wait_ge(s_cev, 4)
            p_kg(3, 1, 3)             # 10: slot3 k3-5 (center)
            # t>=1: steady-state order and gates
            for t in range(1, niter):
                hcur = hb[t % 2]
                v.wait_ge(s_ev, 4 * (t - 1) + 1)
                v.wait_ge(s_pe, 6 * (t - 1) + 1)
                p_s0kg2(t, hcur)
                v.wait_ge(s_ev, 4 * (t - 1) + 2)
                v.wait_ge(s_pe, 6 * (t - 1) + 2)
                p_s1kg01(t, hcur)
                v.wait_ge(s_ev, 4 * (t - 1) + 3)
                p_s1kg2(t, hcur)
                v.wait_ge(s_pe, 6 * (t - 1) + 3)
                p_s0kg1(t, hcur)
                v.wait_ge(s_ev, 4 * (t - 1) + 4)
                v.wait_ge(s_pe, 6 * (t - 1) + 4)
                p_up3(t, hcur)
                if t >= 2:
                    v.wait_ge(s_pe, 6 * (t - 2) + 5)   # pr2 parity WAR
                p_s2k78(t, hcur)
                v.wait_ge(s_pe, 6 * (t - 1) + 6)
                p_s3kg01(t, hcur)
                p_dn3(t, hcur)

        # ---------------- PE: setup sums + iteration tap-sums ----------------
        @block.tensor
        def _(pe):
            pe.wait_ge(s_id, 3)
            # warm-up: ramp the PE p-state past the 3us threshold on dummy
            # 128-col matmuls so the first A-sum runs at full clock
            for _ in range(30):
                pe.matmul(pg(1, 0)[:, 0:128], ident.ap(), ident.ap(),
                          start=True, stop=True, skip_group_check=True)
            # A = sum_ch |g_ch| into psum group 0
            for s in range(NS):
                pe.wait_ge(s_abs, s + 1)
                for ch in range(8):
                    inst = pe.matmul(
                        pg(0, s)[:, 0:512], ident.ap(),
                        gabs.ap()[:, s % 2, ch, 1:513],
                        start=(ch == 0), stop=(ch == 7),
                    )
                    if ch == 7:
                        inst.then_inc(s_apex, 1)
            # center gate: sum of fp16-rounded weights into psum group 1
            for s in range(NS):
                pe.wait_ge(s_gtd, len(DVE_SETUP_KS) * (s + 1))
                pe.wait_ge(s_gtp, len(POOL_SETUP_KS) * (s + 1))
                done = 0
                for k in range(9):
                    if k == 4:
                        continue
                    kg_, kj_ = gk(k)
                    dj = OFFS[k][1]
                    inst = pe.matmul(
                        pg(1, s)[:, 0:512], ident.ap(),
                        gt.ap()[:, kg_, kj_, s, 1 + dj:513 + dj],
                        start=(done == 0), stop=(done == 7),
                    )
                    done += 1
                    if done == 8:
                        inst.then_inc(s_cpe, 1)
            # keepalive: bridge the idle gap between setup and the first
            # window so the p-state ramp survives into T0's matmuls.
            # pg(0,0)'s only setup reader (ln0) is gated done via s_ln.
            pe.wait_ge(s_ln, 1)
            for _ in range(N_KEEPALIVE):
                pe.matmul(pg(0, 0)[:, 0:128], ident.ap(), ident.ap(),
                          start=True, stop=True, skip_group_check=True)
            # iterations: window order 0a'(k678, start psum0), slot1,
            # 0a''(k345), 0b(cross, stop psum0), slot2, slot3;
            # s_pe +1 after each of the 6 groups
            for t in range(niter):
                # t=0 DVE emission order differs; map each group to the
                # s_mul count of its last-needed product
                mw = ([1, 9, 8, 4, 7, 10] if t == 0
                      else [N_T0_MUL + 8 * (t - 1) + 1, N_T0_MUL + 8 * (t - 1) + 3,
                            N_T0_MUL + 8 * (t - 1) + 4, N_T0_MUL + 8 * (t - 1) + 5,
                            N_T0_MUL + 8 * (t - 1) + 6, N_T0_MUL + 8 * (t - 1) + 8])
                groups = [
                    # (psum slot, taps, s_mul wait, s_mulp wait, start, stop)
                    (0, [(k, ident, None) for k in range(6, 9)],
                     mw[0], None, True, False),
                    (1, [(k, ident, None) for k in range(9)],
                     mw[1], None, True, True),
                    (0, [(k, ident, None) for k in range(3, 6)],
                     mw[2], None, False, False),
                    (0, [(k, wup, k) for k in range(3)],
                     mw[3], None, False, True),
                    (2, [(k, ident, None) for k in range(9)],
                     mw[4], 4 * t + 4, True, True),
                    (3, [(k, ident, None) for k in range(6)]
                        + [(k, wdn, 3 + k - 6) for k in range(6, 9)],
                     mw[5], None, True, True),
                ]
                for sg, taps, mw, mpw, st, sp in groups:
                    if mw is not None:
                        pe.wait_ge(s_mul, mw)
                    if mpw is not None:
                        pe.wait_ge(s_mulp, mpw)
                    if st:
                        # psum(sg, t%2) must be drained by its previous reader
                        if t == 0:
                            pe.wait_ge(s_ln, sg + 1)
                        elif t == 1:
                            pe.wait_ge(s_cev, sg + 1)
                        else:
                            pe.wait_ge(s_ev, 4 * (t - 2) + EVPOS[sg])
                    n = len(taps)
                    for j, (k, w_t, crow) in enumerate(taps):
                        kg_, kj_ = gk(k)
                        dj = OFFS[k][1]
                        if sg == 2:
                            mv = pr2.ap()[:, t % 2, kg_, kj_, 1 + dj:513 + dj]
                        elif crow is None:
                            mv = pr.ap()[:, kg_, kj_, sg, 1 + dj:513 + dj]
                        else:
                            mv = prc.ap()[:, crow, 1 + dj:513 + dj]
                        inst = pe.matmul(
                            pg(t % 2, sg)[:, 0:512], w_t.ap(), mv,
                            start=(st and j == 0), stop=(sp and j == n - 1),
                            skip_group_check=True,
                        )
                        if j == n - 1:
                            inst.then_inc(s_pe, 1)

        # ---------------- ACT: ln/exp normalization, evictions ---------------
        @block.scalar
        def _(sc):
            sc.wait_ge(s_hz, 6)   # c_eps ready
            for s in range(NS):
                sc.wait_ge(s_apex, s + 1)
                sc.activation(
                    asb.ap()[:, s, 1:513], pg(0, s)[:, 0:512], AF.Ln,
                    bias=c_eps.ap(),
                ).then_inc(s_ln, 1)
                sc.wait_ge(s_ln, s + 1)
                sc.activation(
                    rr.ap()[:, s, 1:513], asb.ap()[:, s, 1:513], AF.Exp,
                    scale=-1.0,
                ).then_inc(s_rexp, 1)
            for s in range(NS):
                sc.wait_ge(s_cpe, s + 1)
                sc.activation(
                    gt.ap()[:, 1, 1, s, 1:513], pg(1, s)[:, 0:512], AF.Identity,
                    bias=1.0, scale=-1.0,
                ).then_inc(s_cev, 1)
            # eviction order [1, 0, 2, 3] matches PE stop-group order
            pe_stop = {1: 2, 0: 4, 2: 5, 3: 6}
            # DVE cycle-(t-1) readers of h slot sg (last reading instr index)
            war_mul = {0: 8, 1: 2, 2: 7, 3: 7}
            war_mulp = {1: 1, 2: 2, 3: 4}
            for t in range(niter):
                hnext = hb[(t + 1) % 2]
                last = (t == niter - 1)
                for sg in EVORD:
                    sc.wait_ge(s_pe, 6 * t + pe_stop[sg])
                    if t == 0 and sg == EVORD[0]:
                        sc.wait_ge(s_hz, 8)   # h1 pad memsets done
                    if t > 0:
                        # h_next WAR: cycle t-1 readers of this buffer+slot
                        sc.wait_ge(s_mul, N_T0_MUL if t == 1
                                   else N_T0_MUL + 8 * (t - 2) + war_mul[sg])
                        if sg in war_mulp:
                            sc.wait_ge(s_mulp, 4 * (t - 1) + war_mulp[sg])
                    if last:
                        # final eviction straight to fp32 at true magnitude
                        # (undoes the 23 earlier rescales plus this one)
                        sc.activation(
                            asb.ap()[:, sg, 1:513],
                            pg(t % 2, sg)[:, 0:512], AF.Copy,
                            scale=RESCALE * float(1.0 / RESCALE) ** niter,
                        ).then_inc(s_fin, 1)
                    else:
                        sc.activation(
                            hnext.ap()[:, sg, 1:513],
                            pg(t % 2, sg)[:, 0:512], AF.Copy,
                            scale=RESCALE,
                        ).then_inc(s_ev, 1)

        # ---------------- SYNC: gtc shift DMAs + output ----------------------
        @block.sync
        def _(sy):
            # cross weights: one-time partition shifts of the normalized gt
            sy.wait_ge(s_gtd, 3)
            sy.dma_start(gtc.ap()[0:127, 0:3, 0:514],
                         gt.ap()[1:128, 0, :, 0, 0:514]).then_inc(s_gtcu, 16)
            sy.dma_start(gtc.ap()[127:128, 0:3, 0:514],
                         gt.ap()[0:1, 0, :, 0, 0:514]).then_inc(s_gtcu, 16)
            sy.wait_ge(s_gtd, len(DVE_SETUP_KS) * NS)
            sy.wait_ge(s_gtp, len(POOL_SETUP_KS) * NS)
            sy.dma_start(gtc.ap()[1:128, 3:6, 0:514],
                         gt.ap()[0:127, 2, :, 3, 0:514]).then_inc(s_gtcd, 16)
            sy.dma_start(gtc.ap()[0:1, 3:6, 0:514],
                         gt.ap()[127:128, 2, :, 3, 0:514]).then_inc(s_gtcd, 16)
            # per-slot output DMAs, each as soon as its final eviction lands
            for i, sg in enumerate(EVORD):
                sy.wait_ge(s_fin, i + 1)
                o_out = bass.AP(o_dram, sg * W, [[4 * W, 128], [1, W]])
                sy.dma_start(o_out, asb.ap()[:, sg, 1:513]).then_inc(s_out, 16)

    return nc


_NC_CACHE = {}


def kernel(guidance: np.ndarray, x: np.ndarray) -> np.ndarray:
    """guidance [8,8,512,512] f32, x [8,1,512,512] f32 -> [8,1,512,512] f32."""
    guidance = np.ascontiguousarray(np.asarray(guidance, dtype=np.float32))
    x = np.ascontiguousarray(np.asarray(x, dtype=np.float32))
    if "nc" not in _NC_CACHE:
        _NC_CACHE["nc"] = build_program()
    nc = _NC_CACHE["nc"]
    in_maps = [
        {"guidance": guidance[b], "x": x[b].reshape(1, H, W)}
        for b in range(N_CORES)
    ]
    res = run_bass_kernel_spmd(nc, in_maps, core_ids=list(range(N_CORES)))
    out = np.stack([res.results[b]["out"] for b in range(N_CORES)], axis=0)
    return out.astype(np.float32)
